# revision 15
# baseline (speedup 1.0000x reference)
"""Trainium2 Bass kernel for nn_CrossAttention (B=8, N=M=2048, C=512, H=4).

Sharding: data-parallel over batch - one batch element per NeuronCore (8 cores).

v4 design (v3 baseline 228.8us):
  - The 8-core run trips the board GPIO power throttle at ~65us (PE drops
    2.4->~1.95GHz).  v4 cuts total engine activity: gpsimd is eliminated
    entirely (its partition_all_reduce was 58.6us busy/core).
  - Softmax denominator chain per combo: PE ones-matmul column-sum of esE
    into a [1,SW] psum row (512 cyc), DVE reciprocal_approx_fast on the row,
    DMA partition-broadcast of the recip row to [P,SW] (idle DMA engines),
    DVE mul.  Chain k is emitted spread over combo k+1 (j2..j5 slots).
  - pv matmuls run at lag-2 behind the exp (deque), killing the ~300ns
    head-of-queue waits on ACT seen each j in the v3 trace; pairs 6,7 of
    combo k spill into combo k+1's first two j-slots.
  - out-proj weave items moved to j4/j6 slots (one per slot) so the aux
    psum ring (bufs=2) never stalls PE on a back-to-back pair.
  - DMA issue is spread across engine queues (sync: W + FT2 stripe 0 first;
    gpsimd queue: FT1 + FT2 s1-3 + Wp) - v3 serialized 60 issues at ~620ns
    on sync, costing ~9us of startup idle.

Engine budget/core (throttled): PE ~193us busy (pacer), ACT ~154us, DVE ~130us.
If the GPIO throttle lifts with gpsimd gone: PE ~160us.
"""
import sys
from collections import deque

for _p in ("/opt/trn_rl_repo", "/root/.axon_site/_ro/trn_rl_repo"):
    if _p not in sys.path:
        sys.path.insert(0, _p)

import numpy as np
import concourse.bass as bass
import concourse.bacc as bacc
import concourse.tile as tile
from concourse import mybir
from concourse.bass_utils import run_bass_kernel_spmd

F32 = mybir.dt.float32
F16 = mybir.dt.float16
EXP = mybir.ActivationFunctionType.Exp
IDENT = mybir.ActivationFunctionType.Identity

B, N, M, C = 8, 2048, 2048, 512
H, D = 4, 128
SCALE = 1.0 / np.sqrt(C)
P = 128
NB = N // P        # 16 n-blocks
MB = M // P        # 16 m-blocks
KC = C // P        # 4 contraction chunks (also = heads since D=128)
NS = 4             # n-stripes of 512
SW = N // NS       # stripe width 512

# denominator partition-reduction: "pedma" = PE reduce + DMA broadcast
# (gpsimd-free); "gpsimd" = v3's partition_all_reduce fallback
DN_MODE = "pedma"


def build_nc():
    nc = bacc.Bacc(None, target_bir_lowering=False)
    dF1T = nc.dram_tensor("F1T", [C, N], F16, kind="ExternalInput")
    dF2T = nc.dram_tensor("F2T", [C, M], F16, kind="ExternalInput")
    dW = nc.dram_tensor("Wqkv", [C, C], F16, kind="ExternalInput")
    dBqc = nc.dram_tensor("bqc", [P, KC], F32, kind="ExternalInput")
    dWp = nc.dram_tensor("Wproj", [C, C], F16, kind="ExternalInput")
    dBp = nc.dram_tensor("bproj", [1, C], F32, kind="ExternalInput")
    dOut = nc.dram_tensor("OUT", [N, C], F32, kind="ExternalOutput")

    d_ones_col = nc.inline_tensor(np.ones((P, 1), np.float16), name="ones_col")
    d_ident16 = nc.inline_tensor(np.eye(P, dtype=np.float16), name="identity16")

    with tile.TileContext(nc) as tc:
        with (
            tc.tile_pool(name="const", bufs=1) as const,
            tc.tile_pool(name="persist", bufs=1) as persist,
            tc.tile_pool(name="ftp", bufs=1) as ftp,
        ):
            # ---- DMA issue split across engine queues, ONE descriptor per
            # stripe: a [C,*] DRAM tensor maps to a [P, KC, *] SBUF tile via
            # a 3D access pattern, so all 4 kc-chunks land in one issue
            # (~620ns each on the queue; v4 serialized 4x as many). ----
            Wt = const.tile([P, KC, C], F16, name="Wt")
            W = [Wt[:, kc, :] for kc in range(KC)]
            F1t = ftp.tile([P, KC, N], F16, name="F1t")
            FT1 = [F1t[:, kc, :] for kc in range(KC)]
            F2t = ftp.tile([P, KC, M], F16, name="F2t")
            FT2 = [F2t[:, kc, :] for kc in range(KC)]

            def chunked_dram(dt_, width):
                # [C, width] dram AP -> [P, KC, width] (partition-major)
                return dt_.rearrange("(kc p) w -> p kc w", kc=KC, p=P)

            dWv = chunked_dram(dW[:, :], C)
            dF2v = chunked_dram(dF2T[:, :], M)
            dF1v = chunked_dram(dF1T[:, :], N)
            nc.sync.dma_start(Wt, dWv)
            nc.sync.dma_start(F2t[:, :, 0:SW], dF2v[:, :, 0:SW])
            bq_col = const.tile([P, KC], F32)
            nc.sync.dma_start(bq_col, dBqc[:])
            ident16 = const.tile([P, P], F16)
            nc.sync.dma_start(ident16, d_ident16[:])
            nc.sync.dma_start(F2t[:, :, SW:2 * SW], dF2v[:, :, SW:2 * SW])
            ones_col = const.tile([P, 1], F16)
            nc.sync.dma_start(ones_col, d_ones_col[:])
            bp_row = const.tile([1, C], F32)
            nc.sync.dma_start(bp_row, dBp[:])

            # ---- persistent activations ----
            qT = [persist.tile([P, N], F16, name=f"qT{i}") for i in range(KC)]
            kvT = [persist.tile([P, M], F16, name=f"kvT{i}") for i in range(KC)]
            kvn = [persist.tile([P, C], F16, name=f"kvn{i}") for i in range(MB)]

            # Later-needed loads go on the gpsimd queue, gated behind tiny
            # copies that depend on prefix progress (emitted inside the
            # prefix loop below): without the gates, all ~4.5MB streams from
            # t=0 and the startup-critical FT2 stripes crawl at their
            # round-robin share of DMA bandwidth.
            gate = const.tile([1, 2], F16, name="gate")
            Wpt = const.tile([P, KC, C], F16, name="Wpt")
            Wp = [Wpt[:, kc, :] for kc in range(KC)]
            bp_bcast = const.tile([P, C], F32)

            def emit_late_loads(g):
                if g == 0:
                    nc.gpsimd.tensor_copy(gate[0:1, 0:1], kvT[0][0:1, 0:1])
                    for gg in (2, 3):
                        nc.gpsimd.dma_start(
                            F2t[:, :, gg * SW:(gg + 1) * SW],
                            dF2v[:, :, gg * SW:(gg + 1) * SW],
                        )
                elif g == 1:
                    nc.gpsimd.tensor_copy(
                        gate[0:1, 1:2], kvT[0][0:1, SW:SW + 1]
                    )
                    for gg in range(NS):
                        nc.gpsimd.dma_start(
                            F1t[:, :, gg * SW:(gg + 1) * SW],
                            dF1v[:, :, gg * SW:(gg + 1) * SW],
                        )
                    nc.gpsimd.dma_start(Wpt, chunked_dram(dWp[:, :], C))
                    nc.gpsimd.partition_broadcast(bp_bcast, bp_row)

            # ---- prefix: kvT projections + kvn transposes (dense PE) ----
            with tc.tile_pool(name="pfps", bufs=8, space="PSUM") as pfps:
                for g in range(NS):
                    # kvT stripe g for all 4 output chunks
                    for co in range(KC):
                        pj = pfps.tile([P, SW], F32, tag="pj", bufs=4)
                        for kc in range(KC):
                            nc.tensor.matmul(
                                pj,
                                W[kc][:, co * P:(co + 1) * P],
                                FT2[kc][:, g * SW:(g + 1) * SW],
                                start=(kc == 0),
                                stop=(kc == KC - 1),
                            )
                        # evac on ACT (idle in prefix): kvT = pj + bq
                        nc.scalar.activation(
                            kvT[co][:, g * SW:(g + 1) * SW],
                            pj,
                            IDENT,
                            bias=bq_col[:, co:co + 1],
                        )
                        if co == 0:
                            emit_late_loads(g)
                    # kvn for this stripe's 4 m-blocks
                    for mb in range(4 * g, 4 * g + 4):
                        pjt = pfps.tile([P, C], F16, tag="pjt", bufs=2)
                        for hh in range(H):
                            nc.tensor.transpose(
                                pjt[:, hh * P:(hh + 1) * P],
                                kvT[hh][:, mb * P:(mb + 1) * P],
                                ident16,
                            )
                        nc.vector.tensor_copy(kvn[mb], pjt)

            # ---- attention + weaved qT projections + weaved out-proj ----
            with (
                tc.tile_pool(name="xtp", bufs=1) as xtp,
                tc.tile_pool(name="et", bufs=2) as epool,
                tc.tile_pool(name="es", bufs=2) as espool,
                tc.tile_pool(name="scps", bufs=2, space="PSUM") as scps,
                tc.tile_pool(name="pvps", bufs=2, space="PSUM") as pvps,
                tc.tile_pool(name="auxps", bufs=2, space="PSUM") as auxps,
                tc.tile_pool(name="sm", bufs=2) as sm,
                tc.tile_pool(name="osb", bufs=3) as osb,
            ):
                xT = [xtp.tile([P, N], F16, name=f"xT{i}") for i in range(KC)]

                def emit_qT_proj(co, g):
                    pj = auxps.tile([P, SW], F32, tag="aux")
                    for kc in range(KC):
                        nc.tensor.matmul(
                            pj,
                            W[kc][:, co * P:(co + 1) * P],
                            FT1[kc][:, g * SW:(g + 1) * SW],
                            start=(kc == 0),
                            stop=(kc == KC - 1),
                        )
                    nc.vector.tensor_scalar_add(
                        qT[co][:, g * SW:(g + 1) * SW],
                        pj,
                        bq_col[:, co:co + 1],
                    )

                def emit_ph4_start(nb, nchunks=KC):
                    pr = auxps.tile([P, C], F32, tag="aux", name="pr")
                    for kc in range(nchunks):
                        nc.tensor.matmul(
                            pr,
                            xT[kc][:, nb * P:(nb + 1) * P],
                            Wp[kc],
                            start=(kc == 0),
                            stop=(kc == KC - 1),
                        )
                    return pr

                def emit_ph4_finish(nb, pr, kc0=KC):
                    for kc in range(kc0, KC):
                        nc.tensor.matmul(
                            pr,
                            xT[kc][:, nb * P:(nb + 1) * P],
                            Wp[kc],
                            start=False,
                            stop=(kc == KC - 1),
                        )
                    ot = osb.tile([P, C], F32, tag="ot")
                    nc.vector.tensor_add(ot, pr, bp_bcast)
                    nc.sync.dma_start(dOut[nb * P:(nb + 1) * P, :], ot)

                def emit_ph4_nb(nb):
                    emit_ph4_finish(nb, emit_ph4_start(nb))

                combos = [(s, h) for s in range(NS) for h in range(H)]
                # qT-proj weave (j1 slot): combo k emits combo k+1's qT
                qt_sched = [None] * 16
                for k in range(15):
                    qt_sched[k] = combos[k + 1]
                # out-proj weave: stripe s's 4 blocks at combos 4(s+1)+1
                # (j4+j6) and 4(s+1)+2 (j4+j6); stripe 3 in the tail
                op_sched = [[] for _ in range(16)]
                for s in range(NS - 1):
                    for i, nb in enumerate(range(4 * s, 4 * s + 4)):
                        op_sched[4 * (s + 1) + 1 + i // 2].append(nb)

                # deferred normalize chain state from the previous combo
                pending = {}

                def chain_reduce(pp, tail=False):
                    if DN_MODE == "gpsimd":
                        nc.gpsimd.partition_all_reduce(
                            pp["dnb"], pp["esE"], channels=P,
                            reduce_op=bass.bass_isa.ReduceOp.add,
                        )
                        return
                    # in the tail the aux ring is full of out-proj partials;
                    # the freed pv ring hosts the last chain's psum row
                    if tail:
                        ct = pvps.tile([P, SW], F32, tag="pv", name="ct")
                    else:
                        ct = auxps.tile([P, C], F32, tag="aux", name="ct")
                    pp["chain"] = ct
                    nc.tensor.matmul(
                        ct[0:1, 0:SW], ones_col, pp["esE"],
                        start=True, stop=True,
                    )

                def chain_recip(pp):
                    if DN_MODE == "gpsimd":
                        nc.vector.reciprocal_approx_fast(pp["recip"], pp["dnb"])
                        return
                    nc.vector.reciprocal_approx_fast(
                        pp["rrow"], pp["chain"][0:1, 0:SW]
                    )

                def chain_bcast(pp):
                    if DN_MODE == "gpsimd":
                        return
                    # small gpsimd op (~0.7us): 16x cheaper than v3's
                    # partition_all_reduce of the full [P,SW] tile
                    nc.gpsimd.partition_broadcast(pp["bcast"], pp["rrow"])

                def chain_mul(pp):
                    s, h = pp["sh"]
                    mulin = pp["recip"] if DN_MODE == "gpsimd" else pp["bcast"]
                    with nc.allow_low_precision(
                        reason="x values O(0.1); fp16 keeps 5e-4 rel"
                    ):
                        nc.vector.tensor_mul(
                            xT[h][:, s * SW:(s + 1) * SW],
                            pp["pv"], mulin,
                        )

                pvq = deque()
                emit_qT_proj(0, 0)  # combo 0's qT, ahead of the loop

                for k, (s, h) in enumerate(combos):
                    E = epool.tile([P, MB, SW], F16, tag="E")
                    pv = pvps.tile([P, SW], F32, tag="pv")

                    def pv_pair(jj, E=E, pv=pv, h=h):
                        for mb in (2 * jj, 2 * jj + 1):
                            nc.tensor.matmul(
                                pv,
                                kvn[mb][:, h * P:(h + 1) * P],
                                E[:, mb, :],
                                start=(mb == 0),
                                stop=(mb == MB - 1),
                            )

                    esA = espool.tile([P, 4, SW], F16, tag="esA")
                    esB = espool.tile([P, 4, SW], F16, tag="esB")
                    esC = espool.tile([P, 4, SW], F16, tag="esC")
                    esD = espool.tile([P, 2, SW], F16, tag="esD")
                    esE = espool.tile([P, SW], F16, tag="esE")
                    for j in range(MB // 2):
                        sc = scps.tile([P, 2, SW], F32, tag="sc")
                        for i in range(2):
                            mb = 2 * j + i
                            nc.tensor.matmul(
                                sc[:, i, :],
                                kvT[h][:, mb * P:(mb + 1) * P],
                                qT[h][:, s * SW:(s + 1) * SW],
                                start=True,
                                stop=True,
                            )
                        nc.scalar.activation(
                            E[:, 2 * j:2 * j + 2, :].rearrange(
                                "p a b -> p (a b)"
                            ),
                            sc.rearrange("p a b -> p (a b)"),
                            EXP,
                            scale=float(SCALE),
                        )
                        pvq.append(lambda jj=j, pf=pv_pair: pf(jj))
                        if len(pvq) > 2:
                            pvq.popleft()()
                        # weave + deferred chain of the previous combo.
                        # progressive denominator tree: esA@j4, +E[8:12]@j6,
                        # +E[12:16] after the loop, so esE trails the last
                        # exp by only ~2 DVE ops (the v4 bulk tree made the
                        # next combo's chain_reduce stall the PE queue head).
                        if j == 1 and qt_sched[k] is not None:
                            g2, h2 = qt_sched[k][0], qt_sched[k][1]
                            emit_qT_proj(h2, g2)
                        elif j == 3 and pending:
                            chain_reduce(pending)
                        elif j == 4:
                            if pending:
                                chain_recip(pending)
                                chain_bcast(pending)
                            if op_sched[k]:
                                emit_ph4_nb(op_sched[k][0])
                            with nc.allow_low_precision(
                                reason="fp16 partial sums of E for softmax "
                                "denominator; ~1e-3 rel"
                            ):
                                nc.vector.tensor_add(
                                    esA, E[:, 0:4, :], E[:, 4:8, :]
                                )
                        elif j == 5 and pending:
                            chain_mul(pending)
                        elif j == 6:
                            with nc.allow_low_precision(
                                reason="fp16 partial sums of E for softmax "
                                "denominator; ~1e-3 rel"
                            ):
                                nc.vector.tensor_add(esB, esA, E[:, 8:12, :])
                            if len(op_sched[k]) > 1:
                                emit_ph4_nb(op_sched[k][1])
                    with nc.allow_low_precision(
                        reason="fp16 partial sums of E for softmax "
                        "denominator; ~1e-3 rel"
                    ):
                        nc.vector.tensor_add(esC, esB, E[:, 12:16, :])
                        nc.vector.tensor_add(esD, esC[:, 0:2, :], esC[:, 2:4, :])
                        nc.vector.tensor_add(esE, esD[:, 0, :], esD[:, 1, :])
                    pending = {"sh": (s, h), "pv": pv, "esE": esE}
                    if DN_MODE == "gpsimd":
                        pending["dnb"] = sm.tile(
                            [P, SW], F32, tag="dnb", name="dnb"
                        )
                        pending["recip"] = sm.tile(
                            [P, SW], F32, tag="recip", name="recip"
                        )
                    else:
                        pending["rrow"] = sm.tile(
                            [1, SW], F32, tag="rrow", name="rrow"
                        )
                        pending["bcast"] = sm.tile(
                            [P, SW], F32, tag="bcast", name="bcast"
                        )

                # ---- tail: drain pv, last chain, last stripe's out-proj.
                # nb12/nb13's first 3 chunks (heads whose chains are already
                # normalized) fill the PE while the chain-15 latency runs;
                # their h=3 chunks wait on chain_mul.  The chain psum lives
                # in the freed pv ring so both aux banks hold partials. ----
                while pvq:
                    pvq.popleft()()
                pr12 = emit_ph4_start(12, nchunks=KC - 1)
                pr13 = emit_ph4_start(13, nchunks=KC - 1)
                chain_reduce(pending, tail=True)
                chain_recip(pending)
                chain_bcast(pending)
                chain_mul(pending)
                emit_ph4_finish(12, pr12, kc0=KC - 1)
                emit_ph4_finish(13, pr13, kc0=KC - 1)
                for nb in range(14, 4 * NS):
                    emit_ph4_nb(nb)

    nc.compile()
    return nc


_NC = None


def _get_nc():
    global _NC
    if _NC is None:
        _NC = build_nc()
    return _NC


def kernel(F1, F2, W_qkv, b_qkv, W_proj, b_proj, _trace=False):
    F1 = np.asarray(F1)
    F2 = np.asarray(F2)
    F1T = np.ascontiguousarray(
        F1.astype(np.float16).transpose(0, 2, 1)
    )  # [B, C, N]
    F2T = np.ascontiguousarray(F2.astype(np.float16).transpose(0, 2, 1))
    Wh = np.ascontiguousarray(np.asarray(W_qkv).astype(np.float16))
    Wph = np.ascontiguousarray(np.asarray(W_proj).astype(np.float16))
    bqc = np.ascontiguousarray(
        np.asarray(b_qkv, dtype=np.float32).reshape(KC, P).T
    )
    bph = np.ascontiguousarray(
        np.asarray(b_proj, dtype=np.float32).reshape(1, C)
    )

    nc = _get_nc()
    in_maps = [
        {"F1T": F1T[b], "F2T": F2T[b], "Wqkv": Wh, "bqc": bqc,
         "Wproj": Wph, "bproj": bph}
        for b in range(B)
    ]
    res = run_bass_kernel_spmd(
        nc, in_maps, core_ids=list(range(B)), trace=_trace
    )
    out = np.stack([res.results[b]["OUT"] for b in range(B)], axis=0)
    if _trace:
        return out, res
    return out


# revision 16
# speedup vs baseline: 1.1395x; 1.1395x over previous
"""Trainium2 Bass kernel for nn_CrossAttention (B=8, N=M=2048, C=512, H=4).

Sharding: data-parallel over batch - one batch element per NeuronCore (8 cores).

v4 design (v3 baseline 228.8us):
  - The 8-core run trips the board GPIO power throttle at ~65us (PE drops
    2.4->~1.95GHz).  v4 cuts total engine activity: gpsimd is eliminated
    entirely (its partition_all_reduce was 58.6us busy/core).
  - Softmax denominator chain per combo: PE ones-matmul column-sum of esE
    into a [1,SW] psum row (512 cyc), DVE reciprocal_approx_fast on the row,
    DMA partition-broadcast of the recip row to [P,SW] (idle DMA engines),
    DVE mul.  Chain k is emitted spread over combo k+1 (j2..j5 slots).
  - pv matmuls run at lag-2 behind the exp (deque), killing the ~300ns
    head-of-queue waits on ACT seen each j in the v3 trace; pairs 6,7 of
    combo k spill into combo k+1's first two j-slots.
  - out-proj weave items moved to j4/j6 slots (one per slot) so the aux
    psum ring (bufs=2) never stalls PE on a back-to-back pair.
  - DMA issue is spread across engine queues (sync: W + FT2 stripe 0 first;
    gpsimd queue: FT1 + FT2 s1-3 + Wp) - v3 serialized 60 issues at ~620ns
    on sync, costing ~9us of startup idle.

Engine budget/core (throttled): PE ~193us busy (pacer), ACT ~154us, DVE ~130us.
If the GPIO throttle lifts with gpsimd gone: PE ~160us.
"""
import sys
from collections import deque

for _p in ("/opt/trn_rl_repo", "/root/.axon_site/_ro/trn_rl_repo"):
    if _p not in sys.path:
        sys.path.insert(0, _p)

import numpy as np
import concourse.bass as bass
import concourse.bacc as bacc
import concourse.tile as tile
from concourse import mybir
from concourse.bass_utils import run_bass_kernel_spmd

F32 = mybir.dt.float32
F16 = mybir.dt.float16
EXP = mybir.ActivationFunctionType.Exp
IDENT = mybir.ActivationFunctionType.Identity

B, N, M, C = 8, 2048, 2048, 512
H, D = 4, 128
SCALE = 1.0 / np.sqrt(C)
P = 128
NB = N // P        # 16 n-blocks
MB = M // P        # 16 m-blocks
KC = C // P        # 4 contraction chunks (also = heads since D=128)
NS = 4             # n-stripes of 512
SW = N // NS       # stripe width 512

# denominator partition-reduction: "pedma" = PE reduce + DMA broadcast
# (gpsimd-free); "gpsimd" = v3's partition_all_reduce fallback
DN_MODE = "pedma"


def build_nc():
    nc = bacc.Bacc(None, target_bir_lowering=False)
    dF1T = nc.dram_tensor("F1T", [C, N], F16, kind="ExternalInput")
    dF2T = nc.dram_tensor("F2T", [C, M], F16, kind="ExternalInput")
    dW = nc.dram_tensor("Wqkv", [C, C], F16, kind="ExternalInput")
    dBqc = nc.dram_tensor("bqc", [P, KC], F32, kind="ExternalInput")
    dWp = nc.dram_tensor("Wproj", [C, C], F16, kind="ExternalInput")
    dBp = nc.dram_tensor("bproj", [1, C], F32, kind="ExternalInput")
    dOut = nc.dram_tensor("OUT", [N, C], F32, kind="ExternalOutput")

    d_ones_col = nc.inline_tensor(np.ones((P, 1), np.float16), name="ones_col")
    d_ident16 = nc.inline_tensor(np.eye(P, dtype=np.float16), name="identity16")

    with tile.TileContext(nc) as tc:
        with (
            tc.tile_pool(name="const", bufs=1) as const,
            tc.tile_pool(name="persist", bufs=1) as persist,
            tc.tile_pool(name="ftp", bufs=1) as ftp,
        ):
            # ---- DMA issue split across engine queues, ONE descriptor per
            # stripe: a [C,*] DRAM tensor maps to a [P, KC, *] SBUF tile via
            # a 3D access pattern, so all 4 kc-chunks land in one issue
            # (~620ns each on the queue; v4 serialized 4x as many). ----
            Wt = const.tile([P, KC, C], F16, name="Wt")
            W = [Wt[:, kc, :] for kc in range(KC)]
            F1t = ftp.tile([P, KC, N], F16, name="F1t")
            FT1 = [F1t[:, kc, :] for kc in range(KC)]
            F2t = ftp.tile([P, KC, M], F16, name="F2t")
            FT2 = [F2t[:, kc, :] for kc in range(KC)]

            def chunked_dram(dt_, width):
                # [C, width] dram AP -> [P, KC, width] (partition-major)
                return dt_.rearrange("(kc p) w -> p kc w", kc=KC, p=P)

            dWv = chunked_dram(dW[:, :], C)
            dF2v = chunked_dram(dF2T[:, :], M)
            dF1v = chunked_dram(dF1T[:, :], N)
            nc.sync.dma_start(Wt, dWv)
            nc.sync.dma_start(F2t[:, :, 0:SW], dF2v[:, :, 0:SW])
            bq_col = const.tile([P, KC], F32)
            nc.sync.dma_start(bq_col, dBqc[:])
            ident16 = const.tile([P, P], F16)
            nc.sync.dma_start(ident16, d_ident16[:])
            nc.sync.dma_start(F2t[:, :, SW:2 * SW], dF2v[:, :, SW:2 * SW])
            ones_col = const.tile([P, 1], F16)
            nc.sync.dma_start(ones_col, d_ones_col[:])
            bp_row = const.tile([1, C], F32)
            nc.sync.dma_start(bp_row, dBp[:])

            # ---- persistent activations ----
            qT = [persist.tile([P, N], F16, name=f"qT{i}") for i in range(KC)]
            kvT = [persist.tile([P, M], F16, name=f"kvT{i}") for i in range(KC)]
            kvn = [persist.tile([P, C], F16, name=f"kvn{i}") for i in range(MB)]

            # Later-needed loads go on the gpsimd queue, gated behind tiny
            # copies that depend on prefix progress (emitted inside the
            # prefix loop below): without the gates, all ~4.5MB streams from
            # t=0 and the startup-critical FT2 stripes crawl at their
            # round-robin share of DMA bandwidth.
            gate = const.tile([1, 2], F16, name="gate")
            Wpt = const.tile([P, KC, C], F16, name="Wpt")
            Wp = [Wpt[:, kc, :] for kc in range(KC)]
            bp_bcast = const.tile([P, C], F32)

            USE_GATES = False

            def emit_late_loads(g):
                if g == 0:
                    if USE_GATES:
                        nc.gpsimd.tensor_copy(gate[0:1, 0:1], kvT[0][0:1, 0:1])
                    for gg in (2, 3):
                        nc.gpsimd.dma_start(
                            F2t[:, :, gg * SW:(gg + 1) * SW],
                            dF2v[:, :, gg * SW:(gg + 1) * SW],
                        )
                elif g == 1:
                    if USE_GATES:
                        nc.gpsimd.tensor_copy(
                            gate[0:1, 1:2], kvT[0][0:1, SW:SW + 1]
                        )
                    for gg in range(NS):
                        nc.gpsimd.dma_start(
                            F1t[:, :, gg * SW:(gg + 1) * SW],
                            dF1v[:, :, gg * SW:(gg + 1) * SW],
                        )
                    nc.gpsimd.dma_start(Wpt, chunked_dram(dWp[:, :], C))
                    nc.gpsimd.partition_broadcast(bp_bcast, bp_row)

            # ---- prefix: kvT projections + kvn transposes (dense PE) ----
            with tc.tile_pool(name="pfps", bufs=8, space="PSUM") as pfps:
                for g in range(NS):
                    # kvT stripe g for all 4 output chunks
                    for co in range(KC):
                        pj = pfps.tile([P, SW], F32, tag="pj", bufs=4)
                        for kc in range(KC):
                            nc.tensor.matmul(
                                pj,
                                W[kc][:, co * P:(co + 1) * P],
                                FT2[kc][:, g * SW:(g + 1) * SW],
                                start=(kc == 0),
                                stop=(kc == KC - 1),
                            )
                        # evac on ACT (idle in prefix): kvT = pj + bq
                        nc.scalar.activation(
                            kvT[co][:, g * SW:(g + 1) * SW],
                            pj,
                            IDENT,
                            bias=bq_col[:, co:co + 1],
                        )
                        if co == 0:
                            emit_late_loads(g)
                    # kvn for this stripe's 4 m-blocks
                    for mb in range(4 * g, 4 * g + 4):
                        pjt = pfps.tile([P, C], F16, tag="pjt", bufs=2)
                        for hh in range(H):
                            nc.tensor.transpose(
                                pjt[:, hh * P:(hh + 1) * P],
                                kvT[hh][:, mb * P:(mb + 1) * P],
                                ident16,
                            )
                        nc.vector.tensor_copy(kvn[mb], pjt)

            # ---- attention + weaved qT projections + weaved out-proj ----
            with (
                tc.tile_pool(name="xtp", bufs=1) as xtp,
                tc.tile_pool(name="et", bufs=2) as epool,
                tc.tile_pool(name="es", bufs=2) as espool,
                tc.tile_pool(name="scps", bufs=2, space="PSUM") as scps,
                tc.tile_pool(name="pvps", bufs=2, space="PSUM") as pvps,
                tc.tile_pool(name="auxps", bufs=2, space="PSUM") as auxps,
                tc.tile_pool(name="sm", bufs=2) as sm,
                tc.tile_pool(name="osb", bufs=3) as osb,
            ):
                xT = [xtp.tile([P, N], F16, name=f"xT{i}") for i in range(KC)]

                def emit_qT_proj(co, g):
                    pj = auxps.tile([P, SW], F32, tag="aux")
                    for kc in range(KC):
                        nc.tensor.matmul(
                            pj,
                            W[kc][:, co * P:(co + 1) * P],
                            FT1[kc][:, g * SW:(g + 1) * SW],
                            start=(kc == 0),
                            stop=(kc == KC - 1),
                        )
                    nc.vector.tensor_scalar_add(
                        qT[co][:, g * SW:(g + 1) * SW],
                        pj,
                        bq_col[:, co:co + 1],
                    )

                def emit_ph4_start(nb, nchunks=KC):
                    pr = auxps.tile([P, C], F32, tag="aux", name="pr")
                    for kc in range(nchunks):
                        nc.tensor.matmul(
                            pr,
                            xT[kc][:, nb * P:(nb + 1) * P],
                            Wp[kc],
                            start=(kc == 0),
                            stop=(kc == KC - 1),
                        )
                    return pr

                def emit_ph4_finish(nb, pr, kc0=KC):
                    for kc in range(kc0, KC):
                        nc.tensor.matmul(
                            pr,
                            xT[kc][:, nb * P:(nb + 1) * P],
                            Wp[kc],
                            start=False,
                            stop=(kc == KC - 1),
                        )
                    ot = osb.tile([P, C], F32, tag="ot")
                    nc.vector.tensor_add(ot, pr, bp_bcast)
                    nc.sync.dma_start(dOut[nb * P:(nb + 1) * P, :], ot)

                def emit_ph4_nb(nb):
                    emit_ph4_finish(nb, emit_ph4_start(nb))

                combos = [(s, h) for s in range(NS) for h in range(H)]
                # qT-proj weave (j1 slot): combo k emits combo k+1's qT
                qt_sched = [None] * 16
                for k in range(15):
                    qt_sched[k] = combos[k + 1]
                # out-proj weave: stripe s's 4 blocks at combos 4(s+1)+1
                # (j4+j6) and 4(s+1)+2 (j4+j6); stripe 3 in the tail
                op_sched = [[] for _ in range(16)]
                for s in range(NS - 1):
                    for i, nb in enumerate(range(4 * s, 4 * s + 4)):
                        op_sched[4 * (s + 1) + 1 + i // 2].append(nb)

                # deferred normalize chain state from the previous combo
                pending = {}

                def chain_reduce(pp, tail=False):
                    if DN_MODE == "gpsimd":
                        nc.gpsimd.partition_all_reduce(
                            pp["dnb"], pp["esE"], channels=P,
                            reduce_op=bass.bass_isa.ReduceOp.add,
                        )
                        return
                    # in the tail the aux ring is full of out-proj partials;
                    # the freed pv ring hosts the last chain's psum row
                    if tail:
                        ct = pvps.tile([P, SW], F32, tag="pv", name="ct")
                    else:
                        ct = auxps.tile([P, C], F32, tag="aux", name="ct")
                    pp["chain"] = ct
                    nc.tensor.matmul(
                        ct[0:1, 0:SW], ones_col, pp["esE"],
                        start=True, stop=True,
                    )

                def chain_recip(pp):
                    if DN_MODE == "gpsimd":
                        nc.vector.reciprocal_approx_fast(pp["recip"], pp["dnb"])
                        return
                    nc.vector.reciprocal_approx_fast(
                        pp["rrow"], pp["chain"][0:1, 0:SW]
                    )

                def chain_bcast(pp):
                    if DN_MODE == "gpsimd":
                        return
                    # small gpsimd op (~0.7us): 16x cheaper than v3's
                    # partition_all_reduce of the full [P,SW] tile
                    nc.gpsimd.partition_broadcast(pp["bcast"], pp["rrow"])

                def chain_mul(pp):
                    s, h = pp["sh"]
                    mulin = pp["recip"] if DN_MODE == "gpsimd" else pp["bcast"]
                    with nc.allow_low_precision(
                        reason="x values O(0.1); fp16 keeps 5e-4 rel"
                    ):
                        nc.vector.tensor_mul(
                            xT[h][:, s * SW:(s + 1) * SW],
                            pp["pv"], mulin,
                        )

                pvq = deque()
                emit_qT_proj(0, 0)  # combo 0's qT, ahead of the loop

                for k, (s, h) in enumerate(combos):
                    E = epool.tile([P, MB, SW], F16, tag="E")
                    pv = pvps.tile([P, SW], F32, tag="pv")

                    def pv_pair(jj, E=E, pv=pv, h=h):
                        for mb in (2 * jj, 2 * jj + 1):
                            nc.tensor.matmul(
                                pv,
                                kvn[mb][:, h * P:(h + 1) * P],
                                E[:, mb, :],
                                start=(mb == 0),
                                stop=(mb == MB - 1),
                            )

                    esA = espool.tile([P, 4, SW], F16, tag="esA")
                    esB = espool.tile([P, 4, SW], F16, tag="esB")
                    esC = espool.tile([P, 4, SW], F16, tag="esC")
                    esD = espool.tile([P, 2, SW], F16, tag="esD")
                    esE = espool.tile([P, SW], F16, tag="esE")
                    for j in range(MB // 2):
                        sc = scps.tile([P, 2, SW], F32, tag="sc")
                        for i in range(2):
                            mb = 2 * j + i
                            nc.tensor.matmul(
                                sc[:, i, :],
                                kvT[h][:, mb * P:(mb + 1) * P],
                                qT[h][:, s * SW:(s + 1) * SW],
                                start=True,
                                stop=True,
                            )
                        nc.scalar.activation(
                            E[:, 2 * j:2 * j + 2, :].rearrange(
                                "p a b -> p (a b)"
                            ),
                            sc.rearrange("p a b -> p (a b)"),
                            EXP,
                            scale=float(SCALE),
                        )
                        pvq.append(lambda jj=j, pf=pv_pair: pf(jj))
                        if len(pvq) > 2:
                            pvq.popleft()()
                        # weave + deferred chain of the previous combo.
                        # progressive denominator tree: esA@j4, +E[8:12]@j6,
                        # +E[12:16] after the loop, so esE trails the last
                        # exp by only ~2 DVE ops (the v4 bulk tree made the
                        # next combo's chain_reduce stall the PE queue head).
                        if j == 1 and qt_sched[k] is not None:
                            g2, h2 = qt_sched[k][0], qt_sched[k][1]
                            emit_qT_proj(h2, g2)
                        elif j == 3 and pending:
                            chain_reduce(pending)
                        elif j == 4:
                            if pending:
                                chain_recip(pending)
                                chain_bcast(pending)
                            if op_sched[k]:
                                emit_ph4_nb(op_sched[k][0])
                            with nc.allow_low_precision(
                                reason="fp16 partial sums of E for softmax "
                                "denominator; ~1e-3 rel"
                            ):
                                nc.vector.tensor_add(
                                    esA, E[:, 0:4, :], E[:, 4:8, :]
                                )
                        elif j == 5 and pending:
                            chain_mul(pending)
                        elif j == 6:
                            with nc.allow_low_precision(
                                reason="fp16 partial sums of E for softmax "
                                "denominator; ~1e-3 rel"
                            ):
                                nc.vector.tensor_add(esB, esA, E[:, 8:12, :])
                            if len(op_sched[k]) > 1:
                                emit_ph4_nb(op_sched[k][1])
                    with nc.allow_low_precision(
                        reason="fp16 partial sums of E for softmax "
                        "denominator; ~1e-3 rel"
                    ):
                        nc.vector.tensor_add(esC, esB, E[:, 12:16, :])
                        nc.vector.tensor_add(esD, esC[:, 0:2, :], esC[:, 2:4, :])
                        nc.vector.tensor_add(esE, esD[:, 0, :], esD[:, 1, :])
                    pending = {"sh": (s, h), "pv": pv, "esE": esE}
                    if DN_MODE == "gpsimd":
                        pending["dnb"] = sm.tile(
                            [P, SW], F32, tag="dnb", name="dnb"
                        )
                        pending["recip"] = sm.tile(
                            [P, SW], F32, tag="recip", name="recip"
                        )
                    else:
                        pending["rrow"] = sm.tile(
                            [1, SW], F32, tag="rrow", name="rrow"
                        )
                        pending["bcast"] = sm.tile(
                            [P, SW], F32, tag="bcast", name="bcast"
                        )

                # ---- tail: drain pv, last chain, last stripe's out-proj.
                # nb12/nb13's first 3 chunks (heads whose chains are already
                # normalized) fill the PE while the chain-15 latency runs;
                # their h=3 chunks wait on chain_mul.  The chain psum lives
                # in the freed pv ring so both aux banks hold partials. ----
                while pvq:
                    pvq.popleft()()
                pr12 = emit_ph4_start(12, nchunks=KC - 1)
                pr13 = emit_ph4_start(13, nchunks=KC - 1)
                chain_reduce(pending, tail=True)
                chain_recip(pending)
                chain_bcast(pending)
                chain_mul(pending)
                emit_ph4_finish(12, pr12, kc0=KC - 1)
                emit_ph4_finish(13, pr13, kc0=KC - 1)
                for nb in range(14, 4 * NS):
                    emit_ph4_nb(nb)

    nc.compile()
    return nc


_NC = None


def _get_nc():
    global _NC
    if _NC is None:
        _NC = build_nc()
    return _NC


def kernel(F1, F2, W_qkv, b_qkv, W_proj, b_proj, _trace=False):
    F1 = np.asarray(F1)
    F2 = np.asarray(F2)
    F1T = np.ascontiguousarray(
        F1.astype(np.float16).transpose(0, 2, 1)
    )  # [B, C, N]
    F2T = np.ascontiguousarray(F2.astype(np.float16).transpose(0, 2, 1))
    Wh = np.ascontiguousarray(np.asarray(W_qkv).astype(np.float16))
    Wph = np.ascontiguousarray(np.asarray(W_proj).astype(np.float16))
    bqc = np.ascontiguousarray(
        np.asarray(b_qkv, dtype=np.float32).reshape(KC, P).T
    )
    bph = np.ascontiguousarray(
        np.asarray(b_proj, dtype=np.float32).reshape(1, C)
    )

    nc = _get_nc()
    in_maps = [
        {"F1T": F1T[b], "F2T": F2T[b], "Wqkv": Wh, "bqc": bqc,
         "Wproj": Wph, "bproj": bph}
        for b in range(B)
    ]
    res = run_bass_kernel_spmd(
        nc, in_maps, core_ids=list(range(B)), trace=_trace
    )
    out = np.stack([res.results[b]["OUT"] for b in range(B)], axis=0)
    if _trace:
        return out, res
    return out


# revision 18
# speedup vs baseline: 1.1876x; 1.0423x over previous
"""Trainium2 Bass kernel for nn_CrossAttention (B=8, N=M=2048, C=512, H=4).

Sharding: data-parallel over batch - one batch element per NeuronCore (8 cores).

v4 design (v3 baseline 228.8us):
  - The 8-core run trips the board GPIO power throttle at ~65us (PE drops
    2.4->~1.95GHz).  v4 cuts total engine activity: gpsimd is eliminated
    entirely (its partition_all_reduce was 58.6us busy/core).
  - Softmax denominator chain per combo: PE ones-matmul column-sum of esE
    into a [1,SW] psum row (512 cyc), DVE reciprocal_approx_fast on the row,
    DMA partition-broadcast of the recip row to [P,SW] (idle DMA engines),
    DVE mul.  Chain k is emitted spread over combo k+1 (j2..j5 slots).
  - pv matmuls run at lag-2 behind the exp (deque), killing the ~300ns
    head-of-queue waits on ACT seen each j in the v3 trace; pairs 6,7 of
    combo k spill into combo k+1's first two j-slots.
  - out-proj weave items moved to j4/j6 slots (one per slot) so the aux
    psum ring (bufs=2) never stalls PE on a back-to-back pair.
  - DMA issue is spread across engine queues (sync: W + FT2 stripe 0 first;
    gpsimd queue: FT1 + FT2 s1-3 + Wp) - v3 serialized 60 issues at ~620ns
    on sync, costing ~9us of startup idle.

Engine budget/core (throttled): PE ~193us busy (pacer), ACT ~154us, DVE ~130us.
If the GPIO throttle lifts with gpsimd gone: PE ~160us.
"""
import sys
from collections import deque

for _p in ("/opt/trn_rl_repo", "/root/.axon_site/_ro/trn_rl_repo"):
    if _p not in sys.path:
        sys.path.insert(0, _p)

import numpy as np
import concourse.bass as bass
import concourse.bacc as bacc
import concourse.tile as tile
from concourse import mybir
from concourse.bass_utils import run_bass_kernel_spmd

F32 = mybir.dt.float32
F16 = mybir.dt.float16
EXP = mybir.ActivationFunctionType.Exp
IDENT = mybir.ActivationFunctionType.Identity

B, N, M, C = 8, 2048, 2048, 512
H, D = 4, 128
SCALE = 1.0 / np.sqrt(C)
P = 128
NB = N // P        # 16 n-blocks
MB = M // P        # 16 m-blocks
KC = C // P        # 4 contraction chunks (also = heads since D=128)
NS = 4             # n-stripes of 512
SW = N // NS       # stripe width 512

# denominator partition-reduction: "pedma" = PE reduce + DMA broadcast
# (gpsimd-free); "gpsimd" = v3's partition_all_reduce fallback
DN_MODE = "pedma"


def build_nc():
    nc = bacc.Bacc(None, target_bir_lowering=False)
    dF1T = nc.dram_tensor("F1T", [C, N], F16, kind="ExternalInput")
    dF2T = nc.dram_tensor("F2T", [C, M], F16, kind="ExternalInput")
    dW = nc.dram_tensor("Wqkv", [C, C], F16, kind="ExternalInput")
    dBqc = nc.dram_tensor("bqc", [P, KC], F32, kind="ExternalInput")
    dWp = nc.dram_tensor("Wproj", [C, C], F16, kind="ExternalInput")
    dBp = nc.dram_tensor("bproj", [1, C], F32, kind="ExternalInput")
    dOut = nc.dram_tensor("OUT", [N, C], F32, kind="ExternalOutput")

    d_ones_col = nc.inline_tensor(np.ones((P, 1), np.float16), name="ones_col")
    d_ident16 = nc.inline_tensor(np.eye(P, dtype=np.float16), name="identity16")

    with tile.TileContext(nc) as tc:
        with (
            tc.tile_pool(name="const", bufs=1) as const,
            tc.tile_pool(name="persist", bufs=1) as persist,
            tc.tile_pool(name="ftp", bufs=1) as ftp,
        ):
            # ---- DMA issue split across engine queues, ONE descriptor per
            # stripe: a [C,*] DRAM tensor maps to a [P, KC, *] SBUF tile via
            # a 3D access pattern, so all 4 kc-chunks land in one issue
            # (~620ns each on the queue; v4 serialized 4x as many). ----
            Wt = const.tile([P, KC, C], F16, name="Wt")
            W = [Wt[:, kc, :] for kc in range(KC)]
            F1t = ftp.tile([P, KC, N], F16, name="F1t")
            FT1 = [F1t[:, kc, :] for kc in range(KC)]
            F2t = ftp.tile([P, KC, M], F16, name="F2t")
            FT2 = [F2t[:, kc, :] for kc in range(KC)]

            def chunked_dram(dt_, width):
                # [C, width] dram AP -> [P, KC, width] (partition-major)
                return dt_.rearrange("(kc p) w -> p kc w", kc=KC, p=P)

            dWv = chunked_dram(dW[:, :], C)
            dF2v = chunked_dram(dF2T[:, :], M)
            dF1v = chunked_dram(dF1T[:, :], N)
            nc.sync.dma_start(Wt, dWv)
            nc.sync.dma_start(F2t[:, :, 0:SW], dF2v[:, :, 0:SW])
            nc.sync.dma_start(F2t[:, :, SW:2 * SW], dF2v[:, :, SW:2 * SW])
            bq_col = const.tile([P, KC], F32)
            nc.sync.dma_start(bq_col, dBqc[:])
            ident16 = const.tile([P, P], F16)
            nc.sync.dma_start(ident16, d_ident16[:])
            nc.sync.dma_start(F2t[:, :, 2 * SW:3 * SW], dF2v[:, :, 2 * SW:3 * SW])
            nc.sync.dma_start(F2t[:, :, 3 * SW:4 * SW], dF2v[:, :, 3 * SW:4 * SW])
            ones_col = const.tile([P, 1], F16)
            nc.sync.dma_start(ones_col, d_ones_col[:])
            bp_row = const.tile([1, C], F32)
            nc.sync.dma_start(bp_row, dBp[:])

            # ---- persistent activations ----
            qT = [persist.tile([P, N], F16, name=f"qT{i}") for i in range(KC)]
            kvT = [persist.tile([P, M], F16, name=f"kvT{i}") for i in range(KC)]
            kvn = [persist.tile([P, C], F16, name=f"kvn{i}") for i in range(MB)]

            # FT1 goes on the gpsimd queue in per-chunk pieces: the ~550ns
            # per-issue serialization spreads its 2MB of demand over ~9us so
            # it neither starves the startup-critical FT2 stripes (a t=0
            # flood costs ~17us of PE idle) nor lands as one full-bandwidth
            # burst during peak prefix compute (which trips the P0 power
            # downclock for the rest of the run - measured 238us vs 203).
            Wpt = const.tile([P, KC, C], F16, name="Wpt")
            Wp = [Wpt[:, kc, :] for kc in range(KC)]
            bp_bcast = const.tile([P, C], F32)

            def emit_late_loads(g):
                if g != 0:
                    return
                for gg in range(NS):
                    for kc in range(KC):
                        nc.gpsimd.dma_start(
                            F1t[:, kc, gg * SW:(gg + 1) * SW],
                            dF1v[:, kc, gg * SW:(gg + 1) * SW],
                        )
                nc.gpsimd.dma_start(Wpt, chunked_dram(dWp[:, :], C))
                nc.gpsimd.partition_broadcast(bp_bcast, bp_row)

            # ---- prefix: kvT projections + kvn transposes (dense PE) ----
            with tc.tile_pool(name="pfps", bufs=8, space="PSUM") as pfps:
                for g in range(NS):
                    # kvT stripe g for all 4 output chunks
                    for co in range(KC):
                        pj = pfps.tile([P, SW], F32, tag="pj", bufs=4)
                        for kc in range(KC):
                            nc.tensor.matmul(
                                pj,
                                W[kc][:, co * P:(co + 1) * P],
                                FT2[kc][:, g * SW:(g + 1) * SW],
                                start=(kc == 0),
                                stop=(kc == KC - 1),
                            )
                        # evac on ACT (idle in prefix): kvT = pj + bq
                        nc.scalar.activation(
                            kvT[co][:, g * SW:(g + 1) * SW],
                            pj,
                            IDENT,
                            bias=bq_col[:, co:co + 1],
                        )
                        if co == 0:
                            emit_late_loads(g)
                    # kvn for this stripe's 4 m-blocks
                    for mb in range(4 * g, 4 * g + 4):
                        pjt = pfps.tile([P, C], F16, tag="pjt", bufs=2)
                        for hh in range(H):
                            nc.tensor.transpose(
                                pjt[:, hh * P:(hh + 1) * P],
                                kvT[hh][:, mb * P:(mb + 1) * P],
                                ident16,
                            )
                        nc.vector.tensor_copy(kvn[mb], pjt)

            # ---- attention + weaved qT projections + weaved out-proj ----
            with (
                tc.tile_pool(name="xtp", bufs=1) as xtp,
                tc.tile_pool(name="et", bufs=2) as epool,
                tc.tile_pool(name="es", bufs=2) as espool,
                tc.tile_pool(name="scps", bufs=2, space="PSUM") as scps,
                tc.tile_pool(name="pvps", bufs=2, space="PSUM") as pvps,
                tc.tile_pool(name="auxps", bufs=2, space="PSUM") as auxps,
                tc.tile_pool(name="sm", bufs=2) as sm,
                tc.tile_pool(name="osb", bufs=3) as osb,
            ):
                xT = [xtp.tile([P, N], F16, name=f"xT{i}") for i in range(KC)]

                def emit_qT_proj(co, g):
                    pj = auxps.tile([P, SW], F32, tag="aux")
                    for kc in range(KC):
                        nc.tensor.matmul(
                            pj,
                            W[kc][:, co * P:(co + 1) * P],
                            FT1[kc][:, g * SW:(g + 1) * SW],
                            start=(kc == 0),
                            stop=(kc == KC - 1),
                        )
                    nc.vector.tensor_scalar_add(
                        qT[co][:, g * SW:(g + 1) * SW],
                        pj,
                        bq_col[:, co:co + 1],
                    )

                def emit_ph4_start(nb, nchunks=KC):
                    pr = auxps.tile([P, C], F32, tag="aux", name="pr")
                    for kc in range(nchunks):
                        nc.tensor.matmul(
                            pr,
                            xT[kc][:, nb * P:(nb + 1) * P],
                            Wp[kc],
                            start=(kc == 0),
                            stop=(kc == KC - 1),
                        )
                    return pr

                def emit_ph4_finish(nb, pr, kc0=KC):
                    for kc in range(kc0, KC):
                        nc.tensor.matmul(
                            pr,
                            xT[kc][:, nb * P:(nb + 1) * P],
                            Wp[kc],
                            start=False,
                            stop=(kc == KC - 1),
                        )
                    ot = osb.tile([P, C], F32, tag="ot")
                    nc.vector.tensor_add(ot, pr, bp_bcast)
                    nc.sync.dma_start(dOut[nb * P:(nb + 1) * P, :], ot)

                def emit_ph4_nb(nb):
                    emit_ph4_finish(nb, emit_ph4_start(nb))

                combos = [(s, h) for s in range(NS) for h in range(H)]
                # qT-proj weave (j1 slot): combo k emits combo k+1's qT
                qt_sched = [None] * 16
                for k in range(15):
                    qt_sched[k] = combos[k + 1]
                # out-proj weave: stripe s's 4 blocks at combos 4(s+1)+1
                # (j4+j6) and 4(s+1)+2 (j4+j6); stripe 3 in the tail
                op_sched = [[] for _ in range(16)]
                for s in range(NS - 1):
                    for i, nb in enumerate(range(4 * s, 4 * s + 4)):
                        op_sched[4 * (s + 1) + 1 + i // 2].append(nb)

                # deferred normalize chain state from the previous combo
                pending = {}

                def chain_reduce(pp, tail=False):
                    if DN_MODE == "gpsimd":
                        nc.gpsimd.partition_all_reduce(
                            pp["dnb"], pp["esE"], channels=P,
                            reduce_op=bass.bass_isa.ReduceOp.add,
                        )
                        return
                    # in the tail the aux ring is full of out-proj partials;
                    # the freed pv ring hosts the last chain's psum row
                    if tail:
                        ct = pvps.tile([P, SW], F32, tag="pv", name="ct")
                    else:
                        ct = auxps.tile([P, C], F32, tag="aux", name="ct")
                    pp["chain"] = ct
                    nc.tensor.matmul(
                        ct[0:1, 0:SW], ones_col, pp["esE"],
                        start=True, stop=True,
                    )

                def chain_recip(pp):
                    if DN_MODE == "gpsimd":
                        nc.vector.reciprocal_approx_fast(pp["recip"], pp["dnb"])
                        return
                    nc.vector.reciprocal_approx_fast(
                        pp["rrow"], pp["chain"][0:1, 0:SW]
                    )

                def chain_bcast(pp):
                    if DN_MODE == "gpsimd":
                        return
                    # small gpsimd op (~0.7us): 16x cheaper than v3's
                    # partition_all_reduce of the full [P,SW] tile
                    nc.gpsimd.partition_broadcast(pp["bcast"], pp["rrow"])

                def chain_mul(pp):
                    s, h = pp["sh"]
                    mulin = pp["recip"] if DN_MODE == "gpsimd" else pp["bcast"]
                    with nc.allow_low_precision(
                        reason="x values O(0.1); fp16 keeps 5e-4 rel"
                    ):
                        nc.vector.tensor_mul(
                            xT[h][:, s * SW:(s + 1) * SW],
                            pp["pv"], mulin,
                        )

                pvq = deque()
                emit_qT_proj(0, 0)  # combo 0's qT, ahead of the loop

                for k, (s, h) in enumerate(combos):
                    E = epool.tile([P, MB, SW], F16, tag="E")
                    pv = pvps.tile([P, SW], F32, tag="pv")

                    def pv_pair(jj, E=E, pv=pv, h=h):
                        for mb in (2 * jj, 2 * jj + 1):
                            nc.tensor.matmul(
                                pv,
                                kvn[mb][:, h * P:(h + 1) * P],
                                E[:, mb, :],
                                start=(mb == 0),
                                stop=(mb == MB - 1),
                            )

                    esA = espool.tile([P, 4, SW], F16, tag="esA")
                    esB = espool.tile([P, 4, SW], F16, tag="esB")
                    esC = espool.tile([P, 4, SW], F16, tag="esC")
                    esD = espool.tile([P, 2, SW], F16, tag="esD")
                    esE = espool.tile([P, SW], F16, tag="esE")
                    for j in range(MB // 2):
                        sc = scps.tile([P, 2, SW], F32, tag="sc")
                        for i in range(2):
                            mb = 2 * j + i
                            nc.tensor.matmul(
                                sc[:, i, :],
                                kvT[h][:, mb * P:(mb + 1) * P],
                                qT[h][:, s * SW:(s + 1) * SW],
                                start=True,
                                stop=True,
                            )
                        nc.scalar.activation(
                            E[:, 2 * j:2 * j + 2, :].rearrange(
                                "p a b -> p (a b)"
                            ),
                            sc.rearrange("p a b -> p (a b)"),
                            EXP,
                            scale=float(SCALE),
                        )
                        pvq.append(lambda jj=j, pf=pv_pair: pf(jj))
                        if len(pvq) > 2:
                            pvq.popleft()()
                        # weave + deferred chain of the previous combo.
                        # progressive denominator tree: esA@j4, +E[8:12]@j6,
                        # +E[12:16] after the loop, so esE trails the last
                        # exp by only ~2 DVE ops (the v4 bulk tree made the
                        # next combo's chain_reduce stall the PE queue head).
                        if j == 1 and qt_sched[k] is not None:
                            g2, h2 = qt_sched[k][0], qt_sched[k][1]
                            emit_qT_proj(h2, g2)
                        elif j == 3 and pending:
                            chain_reduce(pending)
                        elif j == 4:
                            if pending:
                                chain_recip(pending)
                                chain_bcast(pending)
                            if op_sched[k]:
                                emit_ph4_nb(op_sched[k][0])
                            with nc.allow_low_precision(
                                reason="fp16 partial sums of E for softmax "
                                "denominator; ~1e-3 rel"
                            ):
                                nc.vector.tensor_add(
                                    esA, E[:, 0:4, :], E[:, 4:8, :]
                                )
                        elif j == 5 and pending:
                            chain_mul(pending)
                        elif j == 6:
                            with nc.allow_low_precision(
                                reason="fp16 partial sums of E for softmax "
                                "denominator; ~1e-3 rel"
                            ):
                                nc.vector.tensor_add(esB, esA, E[:, 8:12, :])
                            if len(op_sched[k]) > 1:
                                emit_ph4_nb(op_sched[k][1])
                    with nc.allow_low_precision(
                        reason="fp16 partial sums of E for softmax "
                        "denominator; ~1e-3 rel"
                    ):
                        nc.vector.tensor_add(esC, esB, E[:, 12:16, :])
                        nc.vector.tensor_add(esD, esC[:, 0:2, :], esC[:, 2:4, :])
                        nc.vector.tensor_add(esE, esD[:, 0, :], esD[:, 1, :])
                    pending = {"sh": (s, h), "pv": pv, "esE": esE}
                    if DN_MODE == "gpsimd":
                        pending["dnb"] = sm.tile(
                            [P, SW], F32, tag="dnb", name="dnb"
                        )
                        pending["recip"] = sm.tile(
                            [P, SW], F32, tag="recip", name="recip"
                        )
                    else:
                        pending["rrow"] = sm.tile(
                            [1, SW], F32, tag="rrow", name="rrow"
                        )
                        pending["bcast"] = sm.tile(
                            [P, SW], F32, tag="bcast", name="bcast"
                        )

                # ---- tail: drain pv, last chain, last stripe's out-proj.
                # nb12/nb13's first 3 chunks (heads whose chains are already
                # normalized) fill the PE while the chain-15 latency runs;
                # their h=3 chunks wait on chain_mul.  The chain psum lives
                # in the freed pv ring so both aux banks hold partials. ----
                while pvq:
                    pvq.popleft()()
                pr12 = emit_ph4_start(12, nchunks=KC - 1)
                pr13 = emit_ph4_start(13, nchunks=KC - 1)
                chain_reduce(pending, tail=True)
                chain_recip(pending)
                chain_bcast(pending)
                chain_mul(pending)
                emit_ph4_finish(12, pr12, kc0=KC - 1)
                emit_ph4_finish(13, pr13, kc0=KC - 1)
                for nb in range(14, 4 * NS):
                    emit_ph4_nb(nb)

    nc.compile()
    return nc


_NC = None


def _get_nc():
    global _NC
    if _NC is None:
        _NC = build_nc()
    return _NC


def kernel(F1, F2, W_qkv, b_qkv, W_proj, b_proj, _trace=False):
    F1 = np.asarray(F1)
    F2 = np.asarray(F2)
    F1T = np.ascontiguousarray(
        F1.astype(np.float16).transpose(0, 2, 1)
    )  # [B, C, N]
    F2T = np.ascontiguousarray(F2.astype(np.float16).transpose(0, 2, 1))
    Wh = np.ascontiguousarray(np.asarray(W_qkv).astype(np.float16))
    Wph = np.ascontiguousarray(np.asarray(W_proj).astype(np.float16))
    bqc = np.ascontiguousarray(
        np.asarray(b_qkv, dtype=np.float32).reshape(KC, P).T
    )
    bph = np.ascontiguousarray(
        np.asarray(b_proj, dtype=np.float32).reshape(1, C)
    )

    nc = _get_nc()
    in_maps = [
        {"F1T": F1T[b], "F2T": F2T[b], "Wqkv": Wh, "bqc": bqc,
         "Wproj": Wph, "bproj": bph}
        for b in range(B)
    ]
    res = run_bass_kernel_spmd(
        nc, in_maps, core_ids=list(range(B)), trace=_trace
    )
    out = np.stack([res.results[b]["OUT"] for b in range(B)], axis=0)
    if _trace:
        return out, res
    return out


# revision 29
# speedup vs baseline: 1.1930x; 1.0045x over previous
"""Trainium2 Bass kernel for nn_CrossAttention (B=8, N=M=2048, C=512, H=4).

Sharding: data-parallel over batch - one batch element per NeuronCore (8 cores).

v4 design (v3 baseline 228.8us):
  - The 8-core run trips the board GPIO power throttle at ~65us (PE drops
    2.4->~1.95GHz).  v4 cuts total engine activity: gpsimd is eliminated
    entirely (its partition_all_reduce was 58.6us busy/core).
  - Softmax denominator chain per combo: PE ones-matmul column-sum of esE
    into a [1,SW] psum row (512 cyc), DVE reciprocal_approx_fast on the row,
    DMA partition-broadcast of the recip row to [P,SW] (idle DMA engines),
    DVE mul.  Chain k is emitted spread over combo k+1 (j2..j5 slots).
  - pv matmuls run at lag-2 behind the exp (deque), killing the ~300ns
    head-of-queue waits on ACT seen each j in the v3 trace; pairs 6,7 of
    combo k spill into combo k+1's first two j-slots.
  - out-proj weave items moved to j4/j6 slots (one per slot) so the aux
    psum ring (bufs=2) never stalls PE on a back-to-back pair.
  - DMA issue is spread across engine queues (sync: W + FT2 stripe 0 first;
    gpsimd queue: FT1 + FT2 s1-3 + Wp) - v3 serialized 60 issues at ~620ns
    on sync, costing ~9us of startup idle.

Engine budget/core (throttled): PE ~193us busy (pacer), ACT ~154us, DVE ~130us.
If the GPIO throttle lifts with gpsimd gone: PE ~160us.
"""
import sys
from collections import deque

for _p in ("/opt/trn_rl_repo", "/root/.axon_site/_ro/trn_rl_repo"):
    if _p not in sys.path:
        sys.path.insert(0, _p)

import numpy as np
import concourse.bass as bass
import concourse.bacc as bacc
import concourse.tile as tile
from concourse import mybir
from concourse.bass_utils import run_bass_kernel_spmd

F32 = mybir.dt.float32
F16 = mybir.dt.float16
F8 = mybir.dt.float8e4
EXP = mybir.ActivationFunctionType.Exp
IDENT = mybir.ActivationFunctionType.Identity
DR = mybir.MatmulPerfMode.DoubleRow

B, N, M, C = 8, 2048, 2048, 512
H, D = 4, 128
SCALE = 1.0 / np.sqrt(C)
P = 128
NB = N // P        # 16 n-blocks
MB = M // P        # 16 m-blocks
KC = C // P        # 4 contraction chunks (also = heads since D=128)
NS = 4             # n-stripes of 512
SW = N // NS       # stripe width 512

# denominator partition-reduction: "pedma" = PE reduce + DMA broadcast
# (gpsimd-free); "gpsimd" = v3's partition_all_reduce fallback
DN_MODE = "pedma"
# fp8 (e4m3) E and kv for the attn@kv matmuls via DoubleRow (2x PE rate).
# Numerically viable (measured metric 1.3e-2 vs the 2e-2 gate) but DEAD on
# trn2: DoubleRow uses all 128 PE columns, so its 64-partition output can
# only land at PSUM partition 0 (walrus 's3d3_mm_valid_dst_partition'), and
# reassembling xT[64:128] needs a partition-move (PE/DMA) plus a 9th PSUM
# bank neither of which fits.  Keep False.
PV_FP8 = False


def build_nc():
    nc = bacc.Bacc(None, target_bir_lowering=False)
    dF1T = nc.dram_tensor("F1T", [C, N], F16, kind="ExternalInput")
    dF2T = nc.dram_tensor("F2T", [C, M], F16, kind="ExternalInput")
    dW = nc.dram_tensor("Wqkv", [C, C], F16, kind="ExternalInput")
    dBqc = nc.dram_tensor("bqc", [P, KC], F32, kind="ExternalInput")
    dWp = nc.dram_tensor("Wproj", [C, C], F16, kind="ExternalInput")
    dBp = nc.dram_tensor("bproj", [1, C], F32, kind="ExternalInput")
    dOut = nc.dram_tensor("OUT", [N, C], F32, kind="ExternalOutput")

    d_ones_col = nc.inline_tensor(np.ones((P, 1), np.float16), name="ones_col")
    d_ident16 = nc.inline_tensor(np.eye(P, dtype=np.float16), name="identity16")

    with tile.TileContext(nc) as tc:
        with (
            tc.tile_pool(name="const", bufs=1) as const,
            tc.tile_pool(name="persist", bufs=1) as persist,
            tc.tile_pool(name="ftp", bufs=1) as ftp,
        ):
            # ---- DMA issue split across engine queues, ONE descriptor per
            # stripe: a [C,*] DRAM tensor maps to a [P, KC, *] SBUF tile via
            # a 3D access pattern, so all 4 kc-chunks land in one issue
            # (~620ns each on the queue; v4 serialized 4x as many). ----
            Wt = const.tile([P, KC, C], F16, name="Wt")
            W = [Wt[:, kc, :] for kc in range(KC)]
            F1t = ftp.tile([P, KC, N], F16, name="F1t")
            FT1 = [F1t[:, kc, :] for kc in range(KC)]
            F2t = ftp.tile([P, KC, M], F16, name="F2t")
            FT2 = [F2t[:, kc, :] for kc in range(KC)]

            def chunked_dram(dt_, width):
                # [C, width] dram AP -> [P, KC, width] (partition-major)
                return dt_.rearrange("(kc p) w -> p kc w", kc=KC, p=P)

            dWv = chunked_dram(dW[:, :], C)
            dF2v = chunked_dram(dF2T[:, :], M)
            dF1v = chunked_dram(dF1T[:, :], N)
            nc.sync.dma_start(Wt, dWv)
            nc.sync.dma_start(F2t[:, :, 0:SW], dF2v[:, :, 0:SW])
            nc.sync.dma_start(F2t[:, :, SW:2 * SW], dF2v[:, :, SW:2 * SW])
            bq_col = const.tile([P, KC], F32)
            nc.sync.dma_start(bq_col, dBqc[:])
            ident16 = const.tile([P, P], F16)
            nc.sync.dma_start(ident16, d_ident16[:])
            nc.sync.dma_start(F2t[:, :, 2 * SW:3 * SW], dF2v[:, :, 2 * SW:3 * SW])
            nc.sync.dma_start(F2t[:, :, 3 * SW:4 * SW], dF2v[:, :, 3 * SW:4 * SW])
            ones_col = const.tile([P, 1], F16)
            nc.sync.dma_start(ones_col, d_ones_col[:])
            bp_row = const.tile([1, C], F32)
            nc.sync.dma_start(bp_row, dBp[:])

            # warm the ACT function table at t~0 (ACT is idle): the first
            # Exp otherwise pays the ~1.3us ACT_TABLE_LOAD inside combo 0
            warm_in = const.tile([1, 2], F32, name="warm_in")
            warm_out = const.tile([1, 2], F16, name="warm_out")
            nc.vector.memset(warm_in, 0.0)
            nc.scalar.activation(warm_out, warm_in, EXP)

            # ---- persistent activations ----
            qT = [persist.tile([P, N], F16, name=f"qT{i}") for i in range(KC)]
            kvT = [persist.tile([P, M], F16, name=f"kvT{i}") for i in range(KC)]
            if PV_FP8:
                # kv in m-block PAIRS [m, 2, C] fp8: the DoubleRow stationary
                # layout (dim1 = the two K-tiles of a 256-deep contraction)
                kvn2 = [
                    persist.tile([P, 2, C], F8, name=f"kvn2_{t}")
                    for t in range(MB // 2)
                ]
            else:
                kvn = [
                    persist.tile([P, C], F16, name=f"kvn{i}") for i in range(MB)
                ]

            # FT1 goes on the gpsimd queue in per-chunk pieces: the ~550ns
            # per-issue serialization spreads its 2MB of demand over ~9us so
            # it neither starves the startup-critical FT2 stripes (a t=0
            # flood costs ~17us of PE idle) nor lands as one full-bandwidth
            # burst during peak prefix compute (which trips the P0 power
            # downclock for the rest of the run - measured 238us vs 203).
            Wpt = const.tile([P, KC, C], F16, name="Wpt")
            Wp = [Wpt[:, kc, :] for kc in range(KC)]
            bp_bcast = const.tile([P, C], F32)

            def emit_late_loads(g):
                if g != 0:
                    return
                for gg in range(NS):
                    for kc in range(KC):
                        nc.gpsimd.dma_start(
                            F1t[:, kc, gg * SW:(gg + 1) * SW],
                            dF1v[:, kc, gg * SW:(gg + 1) * SW],
                        )
                nc.gpsimd.dma_start(Wpt, chunked_dram(dWp[:, :], C))
                nc.gpsimd.partition_broadcast(bp_bcast, bp_row)

            # ---- prefix: kvT projections + kvn transposes (dense PE) ----
            with tc.tile_pool(name="pfps", bufs=8, space="PSUM") as pfps:
                for g in range(NS):
                    # kvT stripe g for all 4 output chunks
                    for co in range(KC):
                        pj = pfps.tile([P, SW], F32, tag="pj", bufs=4)
                        for kc in range(KC):
                            nc.tensor.matmul(
                                pj,
                                W[kc][:, co * P:(co + 1) * P],
                                FT2[kc][:, g * SW:(g + 1) * SW],
                                start=(kc == 0),
                                stop=(kc == KC - 1),
                            )
                        # evac on ACT (idle in prefix): kvT = pj + bq
                        nc.scalar.activation(
                            kvT[co][:, g * SW:(g + 1) * SW],
                            pj,
                            IDENT,
                            bias=bq_col[:, co:co + 1],
                        )
                        if co == 0:
                            emit_late_loads(g)
                    # kvn for this stripe's 4 m-blocks
                    for mb in range(4 * g, 4 * g + 4):
                        pjt = pfps.tile([P, C], F16, tag="pjt", bufs=2)
                        for hh in range(H):
                            nc.tensor.transpose(
                                pjt[:, hh * P:(hh + 1) * P],
                                kvT[hh][:, mb * P:(mb + 1) * P],
                                ident16,
                            )
                        if PV_FP8:
                            with nc.allow_low_precision(
                                reason="fp8 kv for DoubleRow pv; metric "
                                "1.3e-2 vs 2e-2 gate (measured end-to-end)"
                            ):
                                nc.vector.tensor_copy(
                                    kvn2[mb // 2][:, mb % 2, :], pjt
                                )
                        else:
                            nc.vector.tensor_copy(kvn[mb], pjt)

            # ---- attention + weaved qT projections + weaved out-proj ----
            with (
                tc.tile_pool(name="xtp", bufs=1) as xtp,
                tc.tile_pool(name="et", bufs=2) as epool,
                tc.tile_pool(name="es", bufs=2) as espool,
                tc.tile_pool(name="scps", bufs=2, space="PSUM") as scps,
                tc.tile_pool(name="pvps", bufs=2, space="PSUM") as pvps,
                tc.tile_pool(name="auxps", bufs=2, space="PSUM") as auxps,
                tc.tile_pool(name="sm", bufs=2) as sm,
                tc.tile_pool(name="osb", bufs=3) as osb,
            ):
                xT = [xtp.tile([P, N], F16, name=f"xT{i}") for i in range(KC)]

                def emit_qT_proj(co, g):
                    pj = auxps.tile([P, SW], F32, tag="aux")
                    for kc in range(KC):
                        nc.tensor.matmul(
                            pj,
                            W[kc][:, co * P:(co + 1) * P],
                            FT1[kc][:, g * SW:(g + 1) * SW],
                            start=(kc == 0),
                            stop=(kc == KC - 1),
                        )
                    nc.vector.tensor_scalar_add(
                        qT[co][:, g * SW:(g + 1) * SW],
                        pj,
                        bq_col[:, co:co + 1],
                    )

                def emit_ph4_start(nb, nchunks=KC):
                    pr = auxps.tile([P, C], F32, tag="aux", name="pr")
                    for kc in range(nchunks):
                        nc.tensor.matmul(
                            pr,
                            xT[kc][:, nb * P:(nb + 1) * P],
                            Wp[kc],
                            start=(kc == 0),
                            stop=(kc == KC - 1),
                        )
                    return pr

                def emit_ph4_finish(nb, pr, kc0=KC):
                    for kc in range(kc0, KC):
                        nc.tensor.matmul(
                            pr,
                            xT[kc][:, nb * P:(nb + 1) * P],
                            Wp[kc],
                            start=False,
                            stop=(kc == KC - 1),
                        )
                    ot = osb.tile([P, C], F32, tag="ot")
                    nc.vector.tensor_add(ot, pr, bp_bcast)
                    nc.sync.dma_start(dOut[nb * P:(nb + 1) * P, :], ot)

                def emit_ph4_nb(nb):
                    emit_ph4_finish(nb, emit_ph4_start(nb))

                combos = [(s, h) for s in range(NS) for h in range(H)]
                # qT-proj weave (j1 slot): combo k emits combo k+1's qT
                qt_sched = [None] * 16
                for k in range(15):
                    qt_sched[k] = combos[k + 1]
                # out-proj weave: stripe s's 4 blocks at combos 4(s+1)+1
                # (j4+j6) and 4(s+1)+2 (j4+j6); stripe 3 in the tail
                op_sched = [[] for _ in range(16)]
                for s in range(NS - 1):
                    for i, nb in enumerate(range(4 * s, 4 * s + 4)):
                        op_sched[4 * (s + 1) + 1 + i // 2].append(nb)

                # deferred normalize chain state from the previous combo
                pending = {}

                def chain_reduce(pp, tail=False):
                    if DN_MODE == "gpsimd":
                        nc.gpsimd.partition_all_reduce(
                            pp["dnb"], pp["esE"], channels=P,
                            reduce_op=bass.bass_isa.ReduceOp.add,
                        )
                        return
                    # in the tail the aux ring is full of out-proj partials;
                    # the freed pv ring hosts the last chain's psum row
                    if tail:
                        ct = pvps.tile([P, SW], F32, tag="pv", name="ct")
                    else:
                        ct = auxps.tile([P, C], F32, tag="aux", name="ct")
                    pp["chain"] = ct
                    nc.tensor.matmul(
                        ct[0:1, 0:SW], ones_col, pp["esE"],
                        start=True, stop=True,
                    )

                def chain_recip(pp):
                    if DN_MODE == "gpsimd":
                        nc.vector.reciprocal_approx_fast(pp["recip"], pp["dnb"])
                        return
                    nc.vector.reciprocal_approx_fast(
                        pp["rrow"], pp["chain"][0:1, 0:SW]
                    )

                def chain_bcast(pp):
                    if DN_MODE == "gpsimd":
                        return
                    # small gpsimd op (~0.7us): 16x cheaper than v3's
                    # partition_all_reduce of the full [P,SW] tile
                    nc.gpsimd.partition_broadcast(pp["bcast"], pp["rrow"])

                def chain_mul(pp):
                    s, h = pp["sh"]
                    mulin = pp["recip"] if DN_MODE == "gpsimd" else pp["bcast"]
                    with nc.allow_low_precision(
                        reason="x values O(0.1); fp16 keeps 5e-4 rel"
                    ):
                        nc.vector.tensor_mul(
                            xT[h][:, s * SW:(s + 1) * SW],
                            pp["pv"], mulin,
                        )

                pvq = deque()
                emit_qT_proj(0, 0)  # combo 0's qT, ahead of the loop

                HP = P // 2  # 64: DoubleRow output-partition limit
                HS = SW // 2  # 256: DoubleRow moving-free limit

                for k, (s, h) in enumerate(combos):
                    E = epool.tile([P, MB, SW], F8 if PV_FP8 else F16, tag="E")
                    pv = pvps.tile([P, SW], F32, tag="pv")

                    if PV_FP8:
                        def pv_pair(jj, E=E, pv=pv, h=h):
                            # 4 DoubleRow matmuls: out quadrant [64, 256],
                            # contraction 256 (m-blocks 2jj, 2jj+1)
                            for dh in range(2):
                                for nh in range(2):
                                    nc.tensor.matmul(
                                        pv[dh * HP:(dh + 1) * HP,
                                           nh * HS:(nh + 1) * HS],
                                        kvn2[jj][:, :,
                                                 h * P + dh * HP:
                                                 h * P + (dh + 1) * HP],
                                        E[:, 2 * jj:2 * jj + 2,
                                          nh * HS:(nh + 1) * HS],
                                        start=(jj == 0),
                                        stop=(jj == MB // 2 - 1),
                                        perf_mode=DR,
                                        tile_position=(0, dh * HP),
                                    )
                    else:
                        def pv_pair(jj, E=E, pv=pv, h=h):
                            for mb in (2 * jj, 2 * jj + 1):
                                nc.tensor.matmul(
                                    pv,
                                    kvn[mb][:, h * P:(h + 1) * P],
                                    E[:, mb, :],
                                    start=(mb == 0),
                                    stop=(mb == MB - 1),
                                )

                    esA = espool.tile([P, 4, SW], F16, tag="esA")
                    esB = espool.tile([P, 4, SW], F16, tag="esB")
                    esC = espool.tile([P, 4, SW], F16, tag="esC")
                    esD = espool.tile([P, 2, SW], F16, tag="esD")
                    esE = espool.tile([P, SW], F16, tag="esE")
                    for j in range(MB // 2):
                        sc = scps.tile([P, 2, SW], F32, tag="sc")
                        for i in range(2):
                            mb = 2 * j + i
                            nc.tensor.matmul(
                                sc[:, i, :],
                                kvT[h][:, mb * P:(mb + 1) * P],
                                qT[h][:, s * SW:(s + 1) * SW],
                                start=True,
                                stop=True,
                            )
                        with nc.allow_low_precision(
                            reason="fp8 E for DoubleRow pv; metric 1.3e-2 "
                            "vs 2e-2 gate (measured end-to-end)"
                        ):
                            nc.scalar.activation(
                                E[:, 2 * j:2 * j + 2, :].rearrange(
                                    "p a b -> p (a b)"
                                ),
                                sc.rearrange("p a b -> p (a b)"),
                                EXP,
                                scale=float(SCALE),
                            )
                        pvq.append(lambda jj=j, pf=pv_pair: pf(jj))
                        if len(pvq) > 2:
                            pvq.popleft()()
                        # weave + deferred chain of the previous combo.
                        # progressive denominator tree: esA@j4, +E[8:12]@j6,
                        # +E[12:16] after the loop, so esE trails the last
                        # exp by only ~2 DVE ops (the v4 bulk tree made the
                        # next combo's chain_reduce stall the PE queue head).
                        if j == 1 and qt_sched[k] is not None:
                            g2, h2 = qt_sched[k][0], qt_sched[k][1]
                            emit_qT_proj(h2, g2)
                        elif j == 3 and pending:
                            chain_reduce(pending)
                        elif j == 4:
                            if pending:
                                chain_recip(pending)
                                chain_bcast(pending)
                            if op_sched[k]:
                                emit_ph4_nb(op_sched[k][0])
                            with nc.allow_low_precision(
                                reason="fp16 partial sums of E for softmax "
                                "denominator; ~1e-3 rel"
                            ):
                                # fp8 inputs lose the DVE 2-byte perf mode;
                                # the first (widest) level goes to gpsimd,
                                # which is otherwise idle in the main loop
                                eng = nc.gpsimd if PV_FP8 else nc.vector
                                eng.tensor_add(
                                    esA, E[:, 0:4, :], E[:, 4:8, :]
                                )
                        elif j == 5 and pending:
                            chain_mul(pending)
                        elif j == 6:
                            with nc.allow_low_precision(
                                reason="fp16 partial sums of E for softmax "
                                "denominator; ~1e-3 rel"
                            ):
                                nc.vector.tensor_add(esB, esA, E[:, 8:12, :])
                            if len(op_sched[k]) > 1:
                                emit_ph4_nb(op_sched[k][1])
                    with nc.allow_low_precision(
                        reason="fp16 partial sums of E for softmax "
                        "denominator; ~1e-3 rel"
                    ):
                        nc.vector.tensor_add(esC, esB, E[:, 12:16, :])
                        nc.vector.tensor_add(esD, esC[:, 0:2, :], esC[:, 2:4, :])
                        nc.vector.tensor_add(esE, esD[:, 0, :], esD[:, 1, :])
                    pending = {"sh": (s, h), "pv": pv, "esE": esE}
                    if DN_MODE == "gpsimd":
                        pending["dnb"] = sm.tile(
                            [P, SW], F32, tag="dnb", name="dnb"
                        )
                        pending["recip"] = sm.tile(
                            [P, SW], F32, tag="recip", name="recip"
                        )
                    else:
                        pending["rrow"] = sm.tile(
                            [1, SW], F32, tag="rrow", name="rrow"
                        )
                        pending["bcast"] = sm.tile(
                            [P, SW], F32, tag="bcast", name="bcast"
                        )

                # ---- tail: drain pv, last chain, last stripe's out-proj.
                # nb12/nb13's first 3 chunks (heads whose chains are already
                # normalized) fill the PE while the chain-15 latency runs;
                # their h=3 chunks wait on chain_mul.  The chain psum lives
                # in the freed pv ring so both aux banks hold partials. ----
                while pvq:
                    pvq.popleft()()
                pr12 = emit_ph4_start(12, nchunks=KC - 1)
                pr13 = emit_ph4_start(13, nchunks=KC - 1)
                chain_reduce(pending, tail=True)
                chain_recip(pending)
                chain_bcast(pending)
                chain_mul(pending)
                emit_ph4_finish(12, pr12, kc0=KC - 1)
                emit_ph4_finish(13, pr13, kc0=KC - 1)
                for nb in range(14, 4 * NS):
                    emit_ph4_nb(nb)

    nc.compile()
    return nc


_NC = None


def _get_nc():
    global _NC
    if _NC is None:
        _NC = build_nc()
    return _NC


def kernel(F1, F2, W_qkv, b_qkv, W_proj, b_proj, _trace=False):
    F1 = np.asarray(F1)
    F2 = np.asarray(F2)
    F1T = np.ascontiguousarray(
        F1.astype(np.float16).transpose(0, 2, 1)
    )  # [B, C, N]
    F2T = np.ascontiguousarray(F2.astype(np.float16).transpose(0, 2, 1))
    Wh = np.ascontiguousarray(np.asarray(W_qkv).astype(np.float16))
    Wph = np.ascontiguousarray(np.asarray(W_proj).astype(np.float16))
    bqc = np.ascontiguousarray(
        np.asarray(b_qkv, dtype=np.float32).reshape(KC, P).T
    )
    bph = np.ascontiguousarray(
        np.asarray(b_proj, dtype=np.float32).reshape(1, C)
    )

    nc = _get_nc()
    in_maps = [
        {"F1T": F1T[b], "F2T": F2T[b], "Wqkv": Wh, "bqc": bqc,
         "Wproj": Wph, "bproj": bph}
        for b in range(B)
    ]
    res = run_bass_kernel_spmd(
        nc, in_maps, core_ids=list(range(B)), trace=_trace
    )
    out = np.stack([res.results[b]["OUT"] for b in range(B)], axis=0)
    if _trace:
        return out, res
    return out


# revision 31
# speedup vs baseline: 1.2132x; 1.0169x over previous
"""Trainium2 Bass kernel for nn_CrossAttention (B=8, N=M=2048, C=512, H=4).

Sharding: data-parallel over batch - one batch element per NeuronCore (8 cores).

v4 design (v3 baseline 228.8us):
  - The 8-core run trips the board GPIO power throttle at ~65us (PE drops
    2.4->~1.95GHz).  v4 cuts total engine activity: gpsimd is eliminated
    entirely (its partition_all_reduce was 58.6us busy/core).
  - Softmax denominator chain per combo: PE ones-matmul column-sum of esE
    into a [1,SW] psum row (512 cyc), DVE reciprocal_approx_fast on the row,
    DMA partition-broadcast of the recip row to [P,SW] (idle DMA engines),
    DVE mul.  Chain k is emitted spread over combo k+1 (j2..j5 slots).
  - pv matmuls run at lag-2 behind the exp (deque), killing the ~300ns
    head-of-queue waits on ACT seen each j in the v3 trace; pairs 6,7 of
    combo k spill into combo k+1's first two j-slots.
  - out-proj weave items moved to j4/j6 slots (one per slot) so the aux
    psum ring (bufs=2) never stalls PE on a back-to-back pair.
  - DMA issue is spread across engine queues (sync: W + FT2 stripe 0 first;
    gpsimd queue: FT1 + FT2 s1-3 + Wp) - v3 serialized 60 issues at ~620ns
    on sync, costing ~9us of startup idle.

Engine budget/core (throttled): PE ~193us busy (pacer), ACT ~154us, DVE ~130us.
If the GPIO throttle lifts with gpsimd gone: PE ~160us.
"""
import sys
from collections import deque

for _p in ("/opt/trn_rl_repo", "/root/.axon_site/_ro/trn_rl_repo"):
    if _p not in sys.path:
        sys.path.insert(0, _p)

import numpy as np
import concourse.bass as bass
import concourse.bacc as bacc
import concourse.tile as tile
from concourse import mybir
from concourse.bass_utils import run_bass_kernel_spmd

F32 = mybir.dt.float32
F16 = mybir.dt.float16
F8 = mybir.dt.float8e4
EXP = mybir.ActivationFunctionType.Exp
IDENT = mybir.ActivationFunctionType.Identity
DR = mybir.MatmulPerfMode.DoubleRow

B, N, M, C = 8, 2048, 2048, 512
H, D = 4, 128
SCALE = 1.0 / np.sqrt(C)
P = 128
NB = N // P        # 16 n-blocks
MB = M // P        # 16 m-blocks
KC = C // P        # 4 contraction chunks (also = heads since D=128)
NS = 4             # n-stripes of 512
SW = N // NS       # stripe width 512

# denominator partition-reduction: "pedma" = PE reduce + DMA broadcast
# (gpsimd-free); "gpsimd" = v3's partition_all_reduce fallback
DN_MODE = "pedma"
# fp8 (e4m3) E and kv for the attn@kv matmuls via DoubleRow (2x PE rate).
# Numerically viable (measured metric 1.3e-2 vs the 2e-2 gate) but DEAD on
# trn2: DoubleRow uses all 128 PE columns, so its 64-partition output can
# only land at PSUM partition 0 (walrus 's3d3_mm_valid_dst_partition'), and
# reassembling xT[64:128] needs a partition-move (PE/DMA) plus a 9th PSUM
# bank neither of which fits.  Keep False.
PV_FP8 = False


def build_nc():
    nc = bacc.Bacc(None, target_bir_lowering=False)
    dF1T = nc.dram_tensor("F1T", [C, N], F16, kind="ExternalInput")
    dF2T = nc.dram_tensor("F2T", [C, M], F16, kind="ExternalInput")
    dW = nc.dram_tensor("Wqkv", [C, C], F16, kind="ExternalInput")
    dBqc = nc.dram_tensor("bqc", [P, KC], F32, kind="ExternalInput")
    dWp = nc.dram_tensor("Wproj", [C, C], F16, kind="ExternalInput")
    dBp = nc.dram_tensor("bproj", [1, C], F32, kind="ExternalInput")
    dOut = nc.dram_tensor("OUT", [N, C], F32, kind="ExternalOutput")

    d_ones_col = nc.inline_tensor(np.ones((P, 1), np.float16), name="ones_col")
    d_ident16 = nc.inline_tensor(np.eye(P, dtype=np.float16), name="identity16")

    with tile.TileContext(nc) as tc:
        with (
            tc.tile_pool(name="const", bufs=1) as const,
            tc.tile_pool(name="persist", bufs=1) as persist,
            tc.tile_pool(name="ftp", bufs=1) as ftp,
        ):
            # ---- DMA issue split across engine queues, ONE descriptor per
            # stripe: a [C,*] DRAM tensor maps to a [P, KC, *] SBUF tile via
            # a 3D access pattern, so all 4 kc-chunks land in one issue
            # (~620ns each on the queue; v4 serialized 4x as many). ----
            Wt = const.tile([P, KC, C], F16, name="Wt")
            W = [Wt[:, kc, :] for kc in range(KC)]
            F1t = ftp.tile([P, KC, N], F16, name="F1t")
            FT1 = [F1t[:, kc, :] for kc in range(KC)]
            F2t = ftp.tile([P, KC, M], F16, name="F2t")
            FT2 = [F2t[:, kc, :] for kc in range(KC)]

            def chunked_dram(dt_, width):
                # [C, width] dram AP -> [P, KC, width] (partition-major)
                return dt_.rearrange("(kc p) w -> p kc w", kc=KC, p=P)

            dWv = chunked_dram(dW[:, :], C)
            dF2v = chunked_dram(dF2T[:, :], M)
            dF1v = chunked_dram(dF1T[:, :], N)
            # split the first loads so kvT-proj chunk 0 can start ~1us sooner
            nc.sync.dma_start(Wt[:, 0:2, :], dWv[:, 0:2, :])
            nc.sync.dma_start(F2t[:, 0:2, 0:SW], dF2v[:, 0:2, 0:SW])
            nc.sync.dma_start(Wt[:, 2:4, :], dWv[:, 2:4, :])
            nc.sync.dma_start(F2t[:, 2:4, 0:SW], dF2v[:, 2:4, 0:SW])
            nc.sync.dma_start(F2t[:, :, SW:2 * SW], dF2v[:, :, SW:2 * SW])
            bq_col = const.tile([P, KC], F32)
            nc.sync.dma_start(bq_col, dBqc[:])
            ident16 = const.tile([P, P], F16)
            nc.sync.dma_start(ident16, d_ident16[:])
            nc.sync.dma_start(F2t[:, :, 2 * SW:3 * SW], dF2v[:, :, 2 * SW:3 * SW])
            nc.sync.dma_start(F2t[:, :, 3 * SW:4 * SW], dF2v[:, :, 3 * SW:4 * SW])
            ones_col = const.tile([P, 1], F16)
            nc.sync.dma_start(ones_col, d_ones_col[:])
            bp_row = const.tile([1, C], F32)
            nc.sync.dma_start(bp_row, dBp[:])

            # warm the ACT function table at t~0 (ACT is idle): the first
            # Exp otherwise pays the ~1.3us ACT_TABLE_LOAD inside combo 0
            warm_in = const.tile([1, 2], F32, name="warm_in")
            warm_out = const.tile([1, 2], F16, name="warm_out")
            nc.vector.memset(warm_in, 0.0)
            nc.scalar.activation(warm_out, warm_in, EXP)

            # ---- persistent activations ----
            qT = [persist.tile([P, N], F16, name=f"qT{i}") for i in range(KC)]
            kvT = [persist.tile([P, M], F16, name=f"kvT{i}") for i in range(KC)]
            if PV_FP8:
                # kv in m-block PAIRS [m, 2, C] fp8: the DoubleRow stationary
                # layout (dim1 = the two K-tiles of a 256-deep contraction)
                kvn2 = [
                    persist.tile([P, 2, C], F8, name=f"kvn2_{t}")
                    for t in range(MB // 2)
                ]
            else:
                kvn = [
                    persist.tile([P, C], F16, name=f"kvn{i}") for i in range(MB)
                ]

            # FT1 goes on the gpsimd queue in 64KB half-chunks: the ~550ns
            # per-issue serialization dribbles its 2MB over ~18us (~115GB/s)
            # so it neither starves the startup-critical FT2 stripes (a t=0
            # flood costs ~17us of PE idle; even per-chunk issue leaves FT1
            # a ~50% bandwidth share that delays stripe 1 by ~5us) nor lands
            # as one full-bandwidth burst during peak prefix compute (which
            # trips the P0 power downclock for the rest of the run -
            # measured 238us vs 203).
            Wpt = const.tile([P, KC, C], F16, name="Wpt")
            Wp = [Wpt[:, kc, :] for kc in range(KC)]
            bp_bcast = const.tile([P, C], F32)

            def emit_late_loads(g):
                if g != 0:
                    return
                HW = SW // 2
                for gg in range(NS):
                    for kc in range(KC):
                        for hh in range(2):
                            sl = slice(gg * SW + hh * HW, gg * SW + (hh + 1) * HW)
                            nc.gpsimd.dma_start(F1t[:, kc, sl], dF1v[:, kc, sl])
                nc.gpsimd.dma_start(Wpt, chunked_dram(dWp[:, :], C))
                nc.gpsimd.partition_broadcast(bp_bcast, bp_row)

            # ---- prefix: kvT projections + kvn transposes (dense PE) ----
            with tc.tile_pool(name="pfps", bufs=8, space="PSUM") as pfps:
                for g in range(NS):
                    # kvT stripe g for all 4 output chunks
                    for co in range(KC):
                        pj = pfps.tile([P, SW], F32, tag="pj", bufs=4)
                        for kc in range(KC):
                            nc.tensor.matmul(
                                pj,
                                W[kc][:, co * P:(co + 1) * P],
                                FT2[kc][:, g * SW:(g + 1) * SW],
                                start=(kc == 0),
                                stop=(kc == KC - 1),
                            )
                        # evac on ACT (idle in prefix): kvT = pj + bq
                        nc.scalar.activation(
                            kvT[co][:, g * SW:(g + 1) * SW],
                            pj,
                            IDENT,
                            bias=bq_col[:, co:co + 1],
                        )
                        if co == 0:
                            emit_late_loads(g)
                    # kvn for this stripe's 4 m-blocks
                    for mb in range(4 * g, 4 * g + 4):
                        pjt = pfps.tile([P, C], F16, tag="pjt", bufs=2)
                        for hh in range(H):
                            nc.tensor.transpose(
                                pjt[:, hh * P:(hh + 1) * P],
                                kvT[hh][:, mb * P:(mb + 1) * P],
                                ident16,
                            )
                        if PV_FP8:
                            with nc.allow_low_precision(
                                reason="fp8 kv for DoubleRow pv; metric "
                                "1.3e-2 vs 2e-2 gate (measured end-to-end)"
                            ):
                                nc.vector.tensor_copy(
                                    kvn2[mb // 2][:, mb % 2, :], pjt
                                )
                        else:
                            nc.vector.tensor_copy(kvn[mb], pjt)

            # ---- attention + weaved qT projections + weaved out-proj ----
            with (
                tc.tile_pool(name="xtp", bufs=1) as xtp,
                tc.tile_pool(name="et", bufs=2) as epool,
                tc.tile_pool(name="es", bufs=2) as espool,
                tc.tile_pool(name="scps", bufs=2, space="PSUM") as scps,
                tc.tile_pool(name="pvps", bufs=2, space="PSUM") as pvps,
                tc.tile_pool(name="auxps", bufs=2, space="PSUM") as auxps,
                tc.tile_pool(name="sm", bufs=2) as sm,
                tc.tile_pool(name="osb", bufs=3) as osb,
            ):
                xT = [xtp.tile([P, N], F16, name=f"xT{i}") for i in range(KC)]

                def emit_qT_proj(co, g):
                    pj = auxps.tile([P, SW], F32, tag="aux")
                    for kc in range(KC):
                        nc.tensor.matmul(
                            pj,
                            W[kc][:, co * P:(co + 1) * P],
                            FT1[kc][:, g * SW:(g + 1) * SW],
                            start=(kc == 0),
                            stop=(kc == KC - 1),
                        )
                    nc.vector.tensor_scalar_add(
                        qT[co][:, g * SW:(g + 1) * SW],
                        pj,
                        bq_col[:, co:co + 1],
                    )

                def emit_ph4_start(nb, nchunks=KC):
                    pr = auxps.tile([P, C], F32, tag="aux", name="pr")
                    for kc in range(nchunks):
                        nc.tensor.matmul(
                            pr,
                            xT[kc][:, nb * P:(nb + 1) * P],
                            Wp[kc],
                            start=(kc == 0),
                            stop=(kc == KC - 1),
                        )
                    return pr

                def emit_ph4_finish(nb, pr, kc0=KC):
                    for kc in range(kc0, KC):
                        nc.tensor.matmul(
                            pr,
                            xT[kc][:, nb * P:(nb + 1) * P],
                            Wp[kc],
                            start=False,
                            stop=(kc == KC - 1),
                        )
                    ot = osb.tile([P, C], F32, tag="ot")
                    nc.vector.tensor_add(ot, pr, bp_bcast)
                    nc.sync.dma_start(dOut[nb * P:(nb + 1) * P, :], ot)

                def emit_ph4_nb(nb):
                    emit_ph4_finish(nb, emit_ph4_start(nb))

                combos = [(s, h) for s in range(NS) for h in range(H)]
                # qT-proj weave (j1 slot): combo k emits combo k+1's qT
                qt_sched = [None] * 16
                for k in range(15):
                    qt_sched[k] = combos[k + 1]
                # out-proj weave: stripe s's 4 blocks at combos 4(s+1)+1
                # (j4+j6) and 4(s+1)+2 (j4+j6); stripe 3 in the tail
                op_sched = [[] for _ in range(16)]
                for s in range(NS - 1):
                    for i, nb in enumerate(range(4 * s, 4 * s + 4)):
                        op_sched[4 * (s + 1) + 1 + i // 2].append(nb)

                # deferred normalize chain state from the previous combo
                pending = {}

                def chain_reduce(pp, tail=False):
                    if DN_MODE == "gpsimd":
                        nc.gpsimd.partition_all_reduce(
                            pp["dnb"], pp["esE"], channels=P,
                            reduce_op=bass.bass_isa.ReduceOp.add,
                        )
                        return
                    # in the tail the aux ring is full of out-proj partials;
                    # the freed pv ring hosts the last chain's psum row
                    if tail:
                        ct = pvps.tile([P, SW], F32, tag="pv", name="ct")
                    else:
                        ct = auxps.tile([P, C], F32, tag="aux", name="ct")
                    pp["chain"] = ct
                    nc.tensor.matmul(
                        ct[0:1, 0:SW], ones_col, pp["esE"],
                        start=True, stop=True,
                    )

                def chain_recip(pp):
                    if DN_MODE == "gpsimd":
                        nc.vector.reciprocal_approx_fast(pp["recip"], pp["dnb"])
                        return
                    nc.vector.reciprocal_approx_fast(
                        pp["rrow"], pp["chain"][0:1, 0:SW]
                    )

                def chain_bcast(pp):
                    if DN_MODE == "gpsimd":
                        return
                    # small gpsimd op (~0.7us): 16x cheaper than v3's
                    # partition_all_reduce of the full [P,SW] tile
                    nc.gpsimd.partition_broadcast(pp["bcast"], pp["rrow"])

                def chain_mul(pp):
                    s, h = pp["sh"]
                    mulin = pp["recip"] if DN_MODE == "gpsimd" else pp["bcast"]
                    with nc.allow_low_precision(
                        reason="x values O(0.1); fp16 keeps 5e-4 rel"
                    ):
                        nc.vector.tensor_mul(
                            xT[h][:, s * SW:(s + 1) * SW],
                            pp["pv"], mulin,
                        )

                pvq = deque()
                emit_qT_proj(0, 0)  # combo 0's qT, ahead of the loop

                HP = P // 2  # 64: DoubleRow output-partition limit
                HS = SW // 2  # 256: DoubleRow moving-free limit

                for k, (s, h) in enumerate(combos):
                    E = epool.tile([P, MB, SW], F8 if PV_FP8 else F16, tag="E")
                    pv = pvps.tile([P, SW], F32, tag="pv")

                    if PV_FP8:
                        def pv_pair(jj, E=E, pv=pv, h=h):
                            # 4 DoubleRow matmuls: out quadrant [64, 256],
                            # contraction 256 (m-blocks 2jj, 2jj+1)
                            for dh in range(2):
                                for nh in range(2):
                                    nc.tensor.matmul(
                                        pv[dh * HP:(dh + 1) * HP,
                                           nh * HS:(nh + 1) * HS],
                                        kvn2[jj][:, :,
                                                 h * P + dh * HP:
                                                 h * P + (dh + 1) * HP],
                                        E[:, 2 * jj:2 * jj + 2,
                                          nh * HS:(nh + 1) * HS],
                                        start=(jj == 0),
                                        stop=(jj == MB // 2 - 1),
                                        perf_mode=DR,
                                        tile_position=(0, dh * HP),
                                    )
                    else:
                        def pv_pair(jj, E=E, pv=pv, h=h):
                            for mb in (2 * jj, 2 * jj + 1):
                                nc.tensor.matmul(
                                    pv,
                                    kvn[mb][:, h * P:(h + 1) * P],
                                    E[:, mb, :],
                                    start=(mb == 0),
                                    stop=(mb == MB - 1),
                                )

                    esA = espool.tile([P, 4, SW], F16, tag="esA")
                    esB = espool.tile([P, 4, SW], F16, tag="esB")
                    esC = espool.tile([P, 4, SW], F16, tag="esC")
                    esD = espool.tile([P, 2, SW], F16, tag="esD")
                    esE = espool.tile([P, SW], F16, tag="esE")
                    for j in range(MB // 2):
                        sc = scps.tile([P, 2, SW], F32, tag="sc")
                        for i in range(2):
                            mb = 2 * j + i
                            nc.tensor.matmul(
                                sc[:, i, :],
                                kvT[h][:, mb * P:(mb + 1) * P],
                                qT[h][:, s * SW:(s + 1) * SW],
                                start=True,
                                stop=True,
                            )
                        with nc.allow_low_precision(
                            reason="fp8 E for DoubleRow pv; metric 1.3e-2 "
                            "vs 2e-2 gate (measured end-to-end)"
                        ):
                            nc.scalar.activation(
                                E[:, 2 * j:2 * j + 2, :].rearrange(
                                    "p a b -> p (a b)"
                                ),
                                sc.rearrange("p a b -> p (a b)"),
                                EXP,
                                scale=float(SCALE),
                            )
                        pvq.append(lambda jj=j, pf=pv_pair: pf(jj))
                        if len(pvq) > 2:
                            pvq.popleft()()
                        # weave + deferred chain of the previous combo.
                        # progressive denominator tree: esA@j4, +E[8:12]@j6,
                        # +E[12:16] after the loop, so esE trails the last
                        # exp by only ~2 DVE ops (the v4 bulk tree made the
                        # next combo's chain_reduce stall the PE queue head).
                        if j == 1 and qt_sched[k] is not None:
                            g2, h2 = qt_sched[k][0], qt_sched[k][1]
                            emit_qT_proj(h2, g2)
                        elif j == 3 and pending:
                            chain_reduce(pending)
                        elif j == 4:
                            if pending:
                                chain_recip(pending)
                                chain_bcast(pending)
                            if op_sched[k]:
                                emit_ph4_nb(op_sched[k][0])
                            with nc.allow_low_precision(
                                reason="fp16 partial sums of E for softmax "
                                "denominator; ~1e-3 rel"
                            ):
                                # fp8 inputs lose the DVE 2-byte perf mode;
                                # the first (widest) level goes to gpsimd,
                                # which is otherwise idle in the main loop
                                eng = nc.gpsimd if PV_FP8 else nc.vector
                                eng.tensor_add(
                                    esA, E[:, 0:4, :], E[:, 4:8, :]
                                )
                        elif j == 5 and pending:
                            chain_mul(pending)
                        elif j == 6:
                            with nc.allow_low_precision(
                                reason="fp16 partial sums of E for softmax "
                                "denominator; ~1e-3 rel"
                            ):
                                nc.vector.tensor_add(esB, esA, E[:, 8:12, :])
                            if len(op_sched[k]) > 1:
                                emit_ph4_nb(op_sched[k][1])
                    with nc.allow_low_precision(
                        reason="fp16 partial sums of E for softmax "
                        "denominator; ~1e-3 rel"
                    ):
                        nc.vector.tensor_add(esC, esB, E[:, 12:16, :])
                        nc.vector.tensor_add(esD, esC[:, 0:2, :], esC[:, 2:4, :])
                        nc.vector.tensor_add(esE, esD[:, 0, :], esD[:, 1, :])
                    pending = {"sh": (s, h), "pv": pv, "esE": esE}
                    if DN_MODE == "gpsimd":
                        pending["dnb"] = sm.tile(
                            [P, SW], F32, tag="dnb", name="dnb"
                        )
                        pending["recip"] = sm.tile(
                            [P, SW], F32, tag="recip", name="recip"
                        )
                    else:
                        pending["rrow"] = sm.tile(
                            [1, SW], F32, tag="rrow", name="rrow"
                        )
                        pending["bcast"] = sm.tile(
                            [P, SW], F32, tag="bcast", name="bcast"
                        )

                # ---- tail: drain pv, last chain, last stripe's out-proj.
                # nb12/nb13's first 3 chunks (heads whose chains are already
                # normalized) fill the PE while the chain-15 latency runs;
                # their h=3 chunks wait on chain_mul.  The chain psum lives
                # in the freed pv ring so both aux banks hold partials. ----
                while pvq:
                    pvq.popleft()()
                pr12 = emit_ph4_start(12, nchunks=KC - 1)
                pr13 = emit_ph4_start(13, nchunks=KC - 1)
                chain_reduce(pending, tail=True)
                chain_recip(pending)
                chain_bcast(pending)
                chain_mul(pending)
                emit_ph4_finish(12, pr12, kc0=KC - 1)
                emit_ph4_finish(13, pr13, kc0=KC - 1)
                for nb in range(14, 4 * NS):
                    emit_ph4_nb(nb)

    nc.compile()
    return nc


_NC = None


def _get_nc():
    global _NC
    if _NC is None:
        _NC = build_nc()
    return _NC


def kernel(F1, F2, W_qkv, b_qkv, W_proj, b_proj, _trace=False):
    F1 = np.asarray(F1)
    F2 = np.asarray(F2)
    F1T = np.ascontiguousarray(
        F1.astype(np.float16).transpose(0, 2, 1)
    )  # [B, C, N]
    F2T = np.ascontiguousarray(F2.astype(np.float16).transpose(0, 2, 1))
    Wh = np.ascontiguousarray(np.asarray(W_qkv).astype(np.float16))
    Wph = np.ascontiguousarray(np.asarray(W_proj).astype(np.float16))
    bqc = np.ascontiguousarray(
        np.asarray(b_qkv, dtype=np.float32).reshape(KC, P).T
    )
    bph = np.ascontiguousarray(
        np.asarray(b_proj, dtype=np.float32).reshape(1, C)
    )

    nc = _get_nc()
    in_maps = [
        {"F1T": F1T[b], "F2T": F2T[b], "Wqkv": Wh, "bqc": bqc,
         "Wproj": Wph, "bproj": bph}
        for b in range(B)
    ]
    res = run_bass_kernel_spmd(
        nc, in_maps, core_ids=list(range(B)), trace=_trace
    )
    out = np.stack([res.results[b]["OUT"] for b in range(B)], axis=0)
    if _trace:
        return out, res
    return out


# revision 34
# speedup vs baseline: 1.2194x; 1.0051x over previous
"""Trainium2 Bass kernel for nn_CrossAttention (B=8, N=M=2048, C=512, H=4).

Sharding: data-parallel over batch - one batch element per NeuronCore (8 cores).

v4 design (v3 baseline 228.8us):
  - The 8-core run trips the board GPIO power throttle at ~65us (PE drops
    2.4->~1.95GHz).  v4 cuts total engine activity: gpsimd is eliminated
    entirely (its partition_all_reduce was 58.6us busy/core).
  - Softmax denominator chain per combo: PE ones-matmul column-sum of esE
    into a [1,SW] psum row (512 cyc), DVE reciprocal_approx_fast on the row,
    DMA partition-broadcast of the recip row to [P,SW] (idle DMA engines),
    DVE mul.  Chain k is emitted spread over combo k+1 (j2..j5 slots).
  - pv matmuls run at lag-2 behind the exp (deque), killing the ~300ns
    head-of-queue waits on ACT seen each j in the v3 trace; pairs 6,7 of
    combo k spill into combo k+1's first two j-slots.
  - out-proj weave items moved to j4/j6 slots (one per slot) so the aux
    psum ring (bufs=2) never stalls PE on a back-to-back pair.
  - DMA issue is spread across engine queues (sync: W + FT2 stripe 0 first;
    gpsimd queue: FT1 + FT2 s1-3 + Wp) - v3 serialized 60 issues at ~620ns
    on sync, costing ~9us of startup idle.

Engine budget/core (throttled): PE ~193us busy (pacer), ACT ~154us, DVE ~130us.
If the GPIO throttle lifts with gpsimd gone: PE ~160us.
"""
import sys
from collections import deque

for _p in ("/opt/trn_rl_repo", "/root/.axon_site/_ro/trn_rl_repo"):
    if _p not in sys.path:
        sys.path.insert(0, _p)

import numpy as np
import concourse.bass as bass
import concourse.bacc as bacc
import concourse.tile as tile
from concourse import mybir
from concourse.bass_utils import run_bass_kernel_spmd

F32 = mybir.dt.float32
F16 = mybir.dt.float16
F8 = mybir.dt.float8e4
EXP = mybir.ActivationFunctionType.Exp
IDENT = mybir.ActivationFunctionType.Identity
DR = mybir.MatmulPerfMode.DoubleRow

B, N, M, C = 8, 2048, 2048, 512
H, D = 4, 128
SCALE = 1.0 / np.sqrt(C)
P = 128
NB = N // P        # 16 n-blocks
MB = M // P        # 16 m-blocks
KC = C // P        # 4 contraction chunks (also = heads since D=128)
NS = 4             # n-stripes of 512
SW = N // NS       # stripe width 512

# denominator partition-reduction: "pedma" = PE reduce + DMA broadcast
# (gpsimd-free); "gpsimd" = v3's partition_all_reduce fallback
DN_MODE = "pedma"
# fp8 (e4m3) E and kv for the attn@kv matmuls via DoubleRow (2x PE rate).
# Numerically viable (measured metric 1.3e-2 vs the 2e-2 gate) but DEAD on
# trn2: DoubleRow uses all 128 PE columns, so its 64-partition output can
# only land at PSUM partition 0 (walrus 's3d3_mm_valid_dst_partition'), and
# reassembling xT[64:128] needs a partition-move (PE/DMA) plus a 9th PSUM
# bank neither of which fits.  Keep False.
PV_FP8 = False


def build_nc():
    nc = bacc.Bacc(None, target_bir_lowering=False)
    dF1T = nc.dram_tensor("F1T", [C, N], F16, kind="ExternalInput")
    dF2T = nc.dram_tensor("F2T", [C, M], F16, kind="ExternalInput")
    dW = nc.dram_tensor("Wqkv", [C, C], F16, kind="ExternalInput")
    dBqc = nc.dram_tensor("bqc", [P, KC], F32, kind="ExternalInput")
    dWp = nc.dram_tensor("Wproj", [C, C], F16, kind="ExternalInput")
    dBp = nc.dram_tensor("bproj", [1, C], F32, kind="ExternalInput")
    dOut = nc.dram_tensor("OUT", [N, C], F32, kind="ExternalOutput")

    d_ones_col = nc.inline_tensor(np.ones((P, 1), np.float16), name="ones_col")
    d_ident16 = nc.inline_tensor(np.eye(P, dtype=np.float16), name="identity16")

    with tile.TileContext(nc) as tc:
        with (
            tc.tile_pool(name="const", bufs=1) as const,
            tc.tile_pool(name="persist", bufs=1) as persist,
            tc.tile_pool(name="ftp", bufs=1) as ftp,
        ):
            # ---- DMA issue split across engine queues, ONE descriptor per
            # stripe: a [C,*] DRAM tensor maps to a [P, KC, *] SBUF tile via
            # a 3D access pattern, so all 4 kc-chunks land in one issue
            # (~620ns each on the queue; v4 serialized 4x as many). ----
            Wt = const.tile([P, KC, C], F16, name="Wt")
            W = [Wt[:, kc, :] for kc in range(KC)]
            F1t = ftp.tile([P, KC, N], F16, name="F1t")
            FT1 = [F1t[:, kc, :] for kc in range(KC)]
            F2t = ftp.tile([P, KC, M], F16, name="F2t")
            FT2 = [F2t[:, kc, :] for kc in range(KC)]

            def chunked_dram(dt_, width):
                # [C, width] dram AP -> [P, KC, width] (partition-major)
                return dt_.rearrange("(kc p) w -> p kc w", kc=KC, p=P)

            dWv = chunked_dram(dW[:, :], C)
            dF2v = chunked_dram(dF2T[:, :], M)
            dF1v = chunked_dram(dF1T[:, :], N)
            # quarter-split the first loads in consumption order: the first
            # kvT-proj matmul needs only W[0] + FT2[0] stripe 0 (256KB), not
            # the full megabyte - worth ~3us at the post-barrier start
            for kc in range(KC):
                nc.sync.dma_start(Wt[:, kc, :], dWv[:, kc, :])
                nc.sync.dma_start(
                    F2t[:, kc, 0:SW], dF2v[:, kc, 0:SW]
                )
            nc.sync.dma_start(F2t[:, :, SW:2 * SW], dF2v[:, :, SW:2 * SW])
            bq_col = const.tile([P, KC], F32)
            nc.sync.dma_start(bq_col, dBqc[:])
            ident16 = const.tile([P, P], F16)
            nc.sync.dma_start(ident16, d_ident16[:])
            nc.sync.dma_start(F2t[:, :, 2 * SW:3 * SW], dF2v[:, :, 2 * SW:3 * SW])
            nc.sync.dma_start(F2t[:, :, 3 * SW:4 * SW], dF2v[:, :, 3 * SW:4 * SW])
            ones_col = const.tile([P, 1], F16)
            nc.sync.dma_start(ones_col, d_ones_col[:])
            bp_row = const.tile([1, C], F32)
            nc.sync.dma_start(bp_row, dBp[:])

            # warm the ACT function table at t~0 (ACT is idle): the first
            # Exp otherwise pays the ~1.3us ACT_TABLE_LOAD inside combo 0
            warm_in = const.tile([1, 2], F32, name="warm_in")
            warm_out = const.tile([1, 2], F16, name="warm_out")
            nc.vector.memset(warm_in, 0.0)
            nc.scalar.activation(warm_out, warm_in, EXP)

            # ---- persistent activations ----
            qT = [persist.tile([P, N], F16, name=f"qT{i}") for i in range(KC)]
            kvT = [persist.tile([P, M], F16, name=f"kvT{i}") for i in range(KC)]
            if PV_FP8:
                # kv in m-block PAIRS [m, 2, C] fp8: the DoubleRow stationary
                # layout (dim1 = the two K-tiles of a 256-deep contraction)
                kvn2 = [
                    persist.tile([P, 2, C], F8, name=f"kvn2_{t}")
                    for t in range(MB // 2)
                ]
            else:
                kvn = [
                    persist.tile([P, C], F16, name=f"kvn{i}") for i in range(MB)
                ]

            # FT1 goes on the gpsimd queue in 64KB half-chunks: the ~550ns
            # per-issue serialization dribbles its 2MB over ~18us (~115GB/s)
            # so it neither starves the startup-critical FT2 stripes (a t=0
            # flood costs ~17us of PE idle; even per-chunk issue leaves FT1
            # a ~50% bandwidth share that delays stripe 1 by ~5us) nor lands
            # as one full-bandwidth burst during peak prefix compute (which
            # trips the P0 power downclock for the rest of the run -
            # measured 238us vs 203).
            Wpt = const.tile([P, KC, C], F16, name="Wpt")
            Wp = [Wpt[:, kc, :] for kc in range(KC)]
            bp_bcast = const.tile([P, C], F32)

            def emit_late_loads(g):
                if g != 0:
                    return
                HW = SW // 2
                for gg in range(NS):
                    for kc in range(KC):
                        for hh in range(2):
                            sl = slice(gg * SW + hh * HW, gg * SW + (hh + 1) * HW)
                            nc.gpsimd.dma_start(F1t[:, kc, sl], dF1v[:, kc, sl])
                nc.gpsimd.dma_start(Wpt, chunked_dram(dWp[:, :], C))
                nc.gpsimd.partition_broadcast(bp_bcast, bp_row)

            # ---- prefix: kvT projections + kvn transposes (dense PE) ----
            with tc.tile_pool(name="pfps", bufs=8, space="PSUM") as pfps:
                for g in range(NS):
                    # kvT stripe g for all 4 output chunks
                    for co in range(KC):
                        pj = pfps.tile([P, SW], F32, tag="pj", bufs=4)
                        for kc in range(KC):
                            nc.tensor.matmul(
                                pj,
                                W[kc][:, co * P:(co + 1) * P],
                                FT2[kc][:, g * SW:(g + 1) * SW],
                                start=(kc == 0),
                                stop=(kc == KC - 1),
                            )
                        # evac on ACT (idle in prefix): kvT = pj + bq
                        nc.scalar.activation(
                            kvT[co][:, g * SW:(g + 1) * SW],
                            pj,
                            IDENT,
                            bias=bq_col[:, co:co + 1],
                        )
                        if co == 0:
                            emit_late_loads(g)
                    # kvn for this stripe's 4 m-blocks
                    for mb in range(4 * g, 4 * g + 4):
                        pjt = pfps.tile([P, C], F16, tag="pjt", bufs=2)
                        for hh in range(H):
                            nc.tensor.transpose(
                                pjt[:, hh * P:(hh + 1) * P],
                                kvT[hh][:, mb * P:(mb + 1) * P],
                                ident16,
                            )
                        if PV_FP8:
                            with nc.allow_low_precision(
                                reason="fp8 kv for DoubleRow pv; metric "
                                "1.3e-2 vs 2e-2 gate (measured end-to-end)"
                            ):
                                nc.vector.tensor_copy(
                                    kvn2[mb // 2][:, mb % 2, :], pjt
                                )
                        else:
                            nc.vector.tensor_copy(kvn[mb], pjt)

            # ---- attention + weaved qT projections + weaved out-proj ----
            with (
                tc.tile_pool(name="xtp", bufs=1) as xtp,
                tc.tile_pool(name="et", bufs=2) as epool,
                tc.tile_pool(name="es", bufs=2) as espool,
                tc.tile_pool(name="scps", bufs=2, space="PSUM") as scps,
                tc.tile_pool(name="pvps", bufs=2, space="PSUM") as pvps,
                tc.tile_pool(name="auxps", bufs=2, space="PSUM") as auxps,
                tc.tile_pool(name="sm", bufs=2) as sm,
                tc.tile_pool(name="osb", bufs=3) as osb,
            ):
                xT = [xtp.tile([P, N], F16, name=f"xT{i}") for i in range(KC)]

                def emit_qT_proj(co, g):
                    pj = auxps.tile([P, SW], F32, tag="aux")
                    for kc in range(KC):
                        nc.tensor.matmul(
                            pj,
                            W[kc][:, co * P:(co + 1) * P],
                            FT1[kc][:, g * SW:(g + 1) * SW],
                            start=(kc == 0),
                            stop=(kc == KC - 1),
                        )
                    nc.vector.tensor_scalar_add(
                        qT[co][:, g * SW:(g + 1) * SW],
                        pj,
                        bq_col[:, co:co + 1],
                    )

                def emit_ph4_start(nb, nchunks=KC, pool=None):
                    if pool is None:
                        pr = auxps.tile([P, C], F32, tag="aux", name="pr")
                    else:
                        # tail only: borrow a freed sc-ring slot (2 banks,
                        # use one slab) so the aux ring stays available
                        pr = pool.tile([P, 2, SW], F32, tag="sc", name="prt")[
                            :, 0, :
                        ]
                    for kc in range(nchunks):
                        nc.tensor.matmul(
                            pr,
                            xT[kc][:, nb * P:(nb + 1) * P],
                            Wp[kc],
                            start=(kc == 0),
                            stop=(kc == KC - 1),
                        )
                    return pr

                def emit_ph4_finish(nb, pr, kc0=KC):
                    for kc in range(kc0, KC):
                        nc.tensor.matmul(
                            pr,
                            xT[kc][:, nb * P:(nb + 1) * P],
                            Wp[kc],
                            start=False,
                            stop=(kc == KC - 1),
                        )
                    ot = osb.tile([P, C], F32, tag="ot")
                    nc.vector.tensor_add(ot, pr, bp_bcast)
                    nc.sync.dma_start(dOut[nb * P:(nb + 1) * P, :], ot)

                def emit_ph4_nb(nb):
                    emit_ph4_finish(nb, emit_ph4_start(nb))

                combos = [(s, h) for s in range(NS) for h in range(H)]
                # qT-proj weave (j1 slot): combo k emits combo k+1's qT
                qt_sched = [None] * 16
                for k in range(15):
                    qt_sched[k] = combos[k + 1]
                # out-proj weave: stripe s's 4 blocks at combos 4(s+1)+1
                # (j4+j6) and 4(s+1)+2 (j4+j6); stripe 3 in the tail
                op_sched = [[] for _ in range(16)]
                for s in range(NS - 1):
                    for i, nb in enumerate(range(4 * s, 4 * s + 4)):
                        op_sched[4 * (s + 1) + 1 + i // 2].append(nb)

                # deferred normalize chain state from the previous combo
                pending = {}

                def chain_reduce(pp, tail=False):
                    if DN_MODE == "gpsimd":
                        nc.gpsimd.partition_all_reduce(
                            pp["dnb"], pp["esE"], channels=P,
                            reduce_op=bass.bass_isa.ReduceOp.add,
                        )
                        return
                    # in the tail the aux ring is full of out-proj partials;
                    # the freed pv ring hosts the last chain's psum row
                    if tail:
                        ct = pvps.tile([P, SW], F32, tag="pv", name="ct")
                    else:
                        ct = auxps.tile([P, C], F32, tag="aux", name="ct")
                    pp["chain"] = ct
                    nc.tensor.matmul(
                        ct[0:1, 0:SW], ones_col, pp["esE"],
                        start=True, stop=True,
                    )

                def chain_recip(pp):
                    if DN_MODE == "gpsimd":
                        nc.vector.reciprocal_approx_fast(pp["recip"], pp["dnb"])
                        return
                    nc.vector.reciprocal_approx_fast(
                        pp["rrow"], pp["chain"][0:1, 0:SW]
                    )

                def chain_bcast(pp):
                    if DN_MODE == "gpsimd":
                        return
                    # small gpsimd op (~0.7us): 16x cheaper than v3's
                    # partition_all_reduce of the full [P,SW] tile
                    nc.gpsimd.partition_broadcast(pp["bcast"], pp["rrow"])

                def chain_mul(pp):
                    s, h = pp["sh"]
                    mulin = pp["recip"] if DN_MODE == "gpsimd" else pp["bcast"]
                    with nc.allow_low_precision(
                        reason="x values O(0.1); fp16 keeps 5e-4 rel"
                    ):
                        nc.vector.tensor_mul(
                            xT[h][:, s * SW:(s + 1) * SW],
                            pp["pv"], mulin,
                        )

                pvq = deque()
                emit_qT_proj(0, 0)  # combo 0's qT, ahead of the loop

                HP = P // 2  # 64: DoubleRow output-partition limit
                HS = SW // 2  # 256: DoubleRow moving-free limit

                for k, (s, h) in enumerate(combos):
                    E = epool.tile([P, MB, SW], F8 if PV_FP8 else F16, tag="E")
                    pv = pvps.tile([P, SW], F32, tag="pv")

                    if PV_FP8:
                        def pv_pair(jj, E=E, pv=pv, h=h):
                            # 4 DoubleRow matmuls: out quadrant [64, 256],
                            # contraction 256 (m-blocks 2jj, 2jj+1)
                            for dh in range(2):
                                for nh in range(2):
                                    nc.tensor.matmul(
                                        pv[dh * HP:(dh + 1) * HP,
                                           nh * HS:(nh + 1) * HS],
                                        kvn2[jj][:, :,
                                                 h * P + dh * HP:
                                                 h * P + (dh + 1) * HP],
                                        E[:, 2 * jj:2 * jj + 2,
                                          nh * HS:(nh + 1) * HS],
                                        start=(jj == 0),
                                        stop=(jj == MB // 2 - 1),
                                        perf_mode=DR,
                                        tile_position=(0, dh * HP),
                                    )
                    else:
                        def pv_pair(jj, E=E, pv=pv, h=h):
                            for mb in (2 * jj, 2 * jj + 1):
                                nc.tensor.matmul(
                                    pv,
                                    kvn[mb][:, h * P:(h + 1) * P],
                                    E[:, mb, :],
                                    start=(mb == 0),
                                    stop=(mb == MB - 1),
                                )

                    esA = espool.tile([P, 4, SW], F16, tag="esA")
                    esB = espool.tile([P, 4, SW], F16, tag="esB")
                    esC = espool.tile([P, 4, SW], F16, tag="esC")
                    esD = espool.tile([P, 2, SW], F16, tag="esD")
                    esE = espool.tile([P, SW], F16, tag="esE")
                    for j in range(MB // 2):
                        sc = scps.tile([P, 2, SW], F32, tag="sc")
                        for i in range(2):
                            mb = 2 * j + i
                            nc.tensor.matmul(
                                sc[:, i, :],
                                kvT[h][:, mb * P:(mb + 1) * P],
                                qT[h][:, s * SW:(s + 1) * SW],
                                start=True,
                                stop=True,
                            )
                        with nc.allow_low_precision(
                            reason="fp8 E for DoubleRow pv; metric 1.3e-2 "
                            "vs 2e-2 gate (measured end-to-end)"
                        ):
                            nc.scalar.activation(
                                E[:, 2 * j:2 * j + 2, :].rearrange(
                                    "p a b -> p (a b)"
                                ),
                                sc.rearrange("p a b -> p (a b)"),
                                EXP,
                                scale=float(SCALE),
                            )
                        pvq.append(lambda jj=j, pf=pv_pair: pf(jj))
                        if len(pvq) > 2:
                            pvq.popleft()()
                        # weave + deferred chain of the previous combo.
                        # progressive denominator tree: esA@j4, +E[8:12]@j6,
                        # +E[12:16] after the loop, so esE trails the last
                        # exp by only ~2 DVE ops (the v4 bulk tree made the
                        # next combo's chain_reduce stall the PE queue head).
                        if j == 1 and qt_sched[k] is not None:
                            g2, h2 = qt_sched[k][0], qt_sched[k][1]
                            emit_qT_proj(h2, g2)
                        elif j == 3 and pending:
                            chain_reduce(pending)
                        elif j == 4:
                            if pending:
                                chain_recip(pending)
                                chain_bcast(pending)
                            if op_sched[k]:
                                emit_ph4_nb(op_sched[k][0])
                            with nc.allow_low_precision(
                                reason="fp16 partial sums of E for softmax "
                                "denominator; ~1e-3 rel"
                            ):
                                # fp8 inputs lose the DVE 2-byte perf mode;
                                # the first (widest) level goes to gpsimd,
                                # which is otherwise idle in the main loop
                                eng = nc.gpsimd if PV_FP8 else nc.vector
                                eng.tensor_add(
                                    esA, E[:, 0:4, :], E[:, 4:8, :]
                                )
                        elif j == 5 and pending:
                            chain_mul(pending)
                        elif j == 6:
                            with nc.allow_low_precision(
                                reason="fp16 partial sums of E for softmax "
                                "denominator; ~1e-3 rel"
                            ):
                                nc.vector.tensor_add(esB, esA, E[:, 8:12, :])
                            if len(op_sched[k]) > 1:
                                emit_ph4_nb(op_sched[k][1])
                    with nc.allow_low_precision(
                        reason="fp16 partial sums of E for softmax "
                        "denominator; ~1e-3 rel"
                    ):
                        nc.vector.tensor_add(esC, esB, E[:, 12:16, :])
                        nc.vector.tensor_add(esD, esC[:, 0:2, :], esC[:, 2:4, :])
                        nc.vector.tensor_add(esE, esD[:, 0, :], esD[:, 1, :])
                    pending = {"sh": (s, h), "pv": pv, "esE": esE}
                    if DN_MODE == "gpsimd":
                        pending["dnb"] = sm.tile(
                            [P, SW], F32, tag="dnb", name="dnb"
                        )
                        pending["recip"] = sm.tile(
                            [P, SW], F32, tag="recip", name="recip"
                        )
                    else:
                        pending["rrow"] = sm.tile(
                            [1, SW], F32, tag="rrow", name="rrow"
                        )
                        pending["bcast"] = sm.tile(
                            [P, SW], F32, tag="bcast", name="bcast"
                        )

                # ---- tail: drain pv, last chain, last stripe's out-proj.
                # nb12/nb13's first 3 chunks (heads whose chains are already
                # normalized) fill the PE while the chain-15 latency runs;
                # their h=3 chunks wait on chain_mul.  The chain psum lives
                # in the freed pv ring so both aux banks hold partials. ----
                while pvq:
                    pvq.popleft()()
                pr12 = emit_ph4_start(12, nchunks=KC - 1)
                pr13 = emit_ph4_start(13, nchunks=KC - 1)
                chain_reduce(pending, tail=True)
                # nb14/nb15 partials borrow the freed sc ring
                pr14 = emit_ph4_start(14, nchunks=KC - 1, pool=scps)
                pr15 = emit_ph4_start(15, nchunks=KC - 1, pool=scps)
                chain_recip(pending)
                chain_bcast(pending)
                chain_mul(pending)
                emit_ph4_finish(12, pr12, kc0=KC - 1)
                emit_ph4_finish(13, pr13, kc0=KC - 1)
                emit_ph4_finish(14, pr14, kc0=KC - 1)
                emit_ph4_finish(15, pr15, kc0=KC - 1)

    nc.compile()
    return nc


_NC = None


def _get_nc():
    global _NC
    if _NC is None:
        _NC = build_nc()
    return _NC


def kernel(F1, F2, W_qkv, b_qkv, W_proj, b_proj, _trace=False):
    F1 = np.asarray(F1)
    F2 = np.asarray(F2)
    F1T = np.ascontiguousarray(
        F1.astype(np.float16).transpose(0, 2, 1)
    )  # [B, C, N]
    F2T = np.ascontiguousarray(F2.astype(np.float16).transpose(0, 2, 1))
    Wh = np.ascontiguousarray(np.asarray(W_qkv).astype(np.float16))
    Wph = np.ascontiguousarray(np.asarray(W_proj).astype(np.float16))
    bqc = np.ascontiguousarray(
        np.asarray(b_qkv, dtype=np.float32).reshape(KC, P).T
    )
    bph = np.ascontiguousarray(
        np.asarray(b_proj, dtype=np.float32).reshape(1, C)
    )

    nc = _get_nc()
    in_maps = [
        {"F1T": F1T[b], "F2T": F2T[b], "Wqkv": Wh, "bqc": bqc,
         "Wproj": Wph, "bproj": bph}
        for b in range(B)
    ]
    res = run_bass_kernel_spmd(
        nc, in_maps, core_ids=list(range(B)), trace=_trace
    )
    out = np.stack([res.results[b]["OUT"] for b in range(B)], axis=0)
    if _trace:
        return out, res
    return out


# revision 38
# speedup vs baseline: 1.2375x; 1.0149x over previous
"""Trainium2 Bass kernel for nn_CrossAttention (B=8, N=M=2048, C=512, H=4).

Sharding: data-parallel over batch - one batch element per NeuronCore (8 cores).

v4 design (v3 baseline 228.8us):
  - The 8-core run trips the board GPIO power throttle at ~65us (PE drops
    2.4->~1.95GHz).  v4 cuts total engine activity: gpsimd is eliminated
    entirely (its partition_all_reduce was 58.6us busy/core).
  - Softmax denominator chain per combo: PE ones-matmul column-sum of esE
    into a [1,SW] psum row (512 cyc), DVE reciprocal_approx_fast on the row,
    DMA partition-broadcast of the recip row to [P,SW] (idle DMA engines),
    DVE mul.  Chain k is emitted spread over combo k+1 (j2..j5 slots).
  - pv matmuls run at lag-2 behind the exp (deque), killing the ~300ns
    head-of-queue waits on ACT seen each j in the v3 trace; pairs 6,7 of
    combo k spill into combo k+1's first two j-slots.
  - out-proj weave items moved to j4/j6 slots (one per slot) so the aux
    psum ring (bufs=2) never stalls PE on a back-to-back pair.
  - DMA issue is spread across engine queues (sync: W + FT2 stripe 0 first;
    gpsimd queue: FT1 + FT2 s1-3 + Wp) - v3 serialized 60 issues at ~620ns
    on sync, costing ~9us of startup idle.

Engine budget/core (throttled): PE ~193us busy (pacer), ACT ~154us, DVE ~130us.
If the GPIO throttle lifts with gpsimd gone: PE ~160us.
"""
import sys
from collections import deque

for _p in ("/opt/trn_rl_repo", "/root/.axon_site/_ro/trn_rl_repo"):
    if _p not in sys.path:
        sys.path.insert(0, _p)

import numpy as np
import concourse.bass as bass
import concourse.bacc as bacc
import concourse.tile as tile
from concourse import mybir
from concourse.bass_utils import run_bass_kernel_spmd

F32 = mybir.dt.float32
F16 = mybir.dt.float16
F8 = mybir.dt.float8e4
EXP = mybir.ActivationFunctionType.Exp
IDENT = mybir.ActivationFunctionType.Identity
DR = mybir.MatmulPerfMode.DoubleRow

B, N, M, C = 8, 2048, 2048, 512
H, D = 4, 128
SCALE = 1.0 / np.sqrt(C)
P = 128
NB = N // P        # 16 n-blocks
MB = M // P        # 16 m-blocks
KC = C // P        # 4 contraction chunks (also = heads since D=128)
NS = 4             # n-stripes of 512
SW = N // NS       # stripe width 512

# denominator partition-reduction: "pedma" = PE reduce + DMA broadcast
# (gpsimd-free); "gpsimd" = v3's partition_all_reduce fallback
DN_MODE = "pedma"
# fp8 (e4m3) E and kv for the attn@kv matmuls via DoubleRow (2x PE rate).
# Numerically viable (measured metric 1.3e-2 vs the 2e-2 gate) but DEAD on
# trn2: DoubleRow uses all 128 PE columns, so its 64-partition output can
# only land at PSUM partition 0 (walrus 's3d3_mm_valid_dst_partition'), and
# reassembling xT[64:128] needs a partition-move (PE/DMA) plus a 9th PSUM
# bank neither of which fits.  Keep False.
PV_FP8 = False


def build_nc():
    nc = bacc.Bacc(None, target_bir_lowering=False)
    dF1T = nc.dram_tensor("F1T", [C, N], F16, kind="ExternalInput")
    dF2T = nc.dram_tensor("F2T", [C, M], F16, kind="ExternalInput")
    dW = nc.dram_tensor("Wqkv", [C, C], F16, kind="ExternalInput")
    dBqc = nc.dram_tensor("bqc", [P, KC], F32, kind="ExternalInput")
    dWp = nc.dram_tensor("Wproj", [C, C], F16, kind="ExternalInput")
    dBp = nc.dram_tensor("bproj", [1, C], F32, kind="ExternalInput")
    dOut = nc.dram_tensor("OUT", [N, C], F32, kind="ExternalOutput")

    d_ones_col = nc.inline_tensor(np.ones((P, 1), np.float16), name="ones_col")
    d_ident16 = nc.inline_tensor(np.eye(P, dtype=np.float16), name="identity16")

    with tile.TileContext(nc) as tc:
        with (
            tc.tile_pool(name="const", bufs=1) as const,
            tc.tile_pool(name="persist", bufs=1) as persist,
            tc.tile_pool(name="ftp", bufs=1) as ftp,
        ):
            # ---- DMA issue split across engine queues, ONE descriptor per
            # stripe: a [C,*] DRAM tensor maps to a [P, KC, *] SBUF tile via
            # a 3D access pattern, so all 4 kc-chunks land in one issue
            # (~620ns each on the queue; v4 serialized 4x as many). ----
            Wt = const.tile([P, KC, C], F16, name="Wt")
            W = [Wt[:, kc, :] for kc in range(KC)]
            F1t = ftp.tile([P, KC, N], F16, name="F1t")
            FT1 = [F1t[:, kc, :] for kc in range(KC)]
            F2t = ftp.tile([P, KC, M], F16, name="F2t")
            FT2 = [F2t[:, kc, :] for kc in range(KC)]

            def chunked_dram(dt_, width):
                # [C, width] dram AP -> [P, KC, width] (partition-major)
                return dt_.rearrange("(kc p) w -> p kc w", kc=KC, p=P)

            dWv = chunked_dram(dW[:, :], C)
            dF2v = chunked_dram(dF2T[:, :], M)
            dF1v = chunked_dram(dF1T[:, :], N)
            # Startup-critical loads spread across engine queues: DMAs on
            # ONE queue transfer sequentially, so the pieces the first
            # matmuls need are split by consumption order across sync
            # (W chunks), vector (FT2 kc0/kc1 + stripes 1-3) and scalar
            # (FT2 kc2/kc3).  Tiny consts ride the gpsimd queue ahead of
            # the FT1 dribble.
            bq_col = const.tile([P, KC], F32)
            ident16 = const.tile([P, P], F16)
            ones_col = const.tile([P, 1], F16)
            bp_row = const.tile([1, C], F32)
            warm_in = const.tile([1, 2], F32, name="warm_in")
            warm_out = const.tile([1, 2], F16, name="warm_out")

            nc.vector.memset(warm_in, 0.0)
            for kc in range(KC):
                nc.sync.dma_start(Wt[:, kc, :], dWv[:, kc, :])
            nc.scalar.dma_start(F2t[:, 0, 0:SW], dF2v[:, 0, 0:SW])
            nc.scalar.dma_start(F2t[:, 1, 0:SW], dF2v[:, 1, 0:SW])
            nc.gpsimd.dma_start(F2t[:, 2, 0:SW], dF2v[:, 2, 0:SW])
            nc.gpsimd.dma_start(F2t[:, 3, 0:SW], dF2v[:, 3, 0:SW])
            nc.gpsimd.dma_start(bq_col, dBqc[:])
            nc.gpsimd.dma_start(ident16, d_ident16[:])
            # warm the ACT function table (after its two DMA issues): the
            # first Exp otherwise pays the ~1.3us ACT_TABLE_LOAD in combo 0
            nc.scalar.activation(warm_out, warm_in, EXP)
            nc.sync.dma_start(F2t[:, :, SW:2 * SW], dF2v[:, :, SW:2 * SW])
            nc.sync.dma_start(F2t[:, :, 2 * SW:3 * SW], dF2v[:, :, 2 * SW:3 * SW])
            nc.sync.dma_start(F2t[:, :, 3 * SW:4 * SW], dF2v[:, :, 3 * SW:4 * SW])

            # ---- persistent activations ----
            qT = [persist.tile([P, N], F16, name=f"qT{i}") for i in range(KC)]
            kvT = [persist.tile([P, M], F16, name=f"kvT{i}") for i in range(KC)]
            if PV_FP8:
                # kv in m-block PAIRS [m, 2, C] fp8: the DoubleRow stationary
                # layout (dim1 = the two K-tiles of a 256-deep contraction)
                kvn2 = [
                    persist.tile([P, 2, C], F8, name=f"kvn2_{t}")
                    for t in range(MB // 2)
                ]
            else:
                kvn = [
                    persist.tile([P, C], F16, name=f"kvn{i}") for i in range(MB)
                ]

            # FT1 goes on the gpsimd queue in 64KB half-chunks: the ~550ns
            # per-issue serialization dribbles its 2MB over ~18us (~115GB/s)
            # so it neither starves the startup-critical FT2 stripes (a t=0
            # flood costs ~17us of PE idle; even per-chunk issue leaves FT1
            # a ~50% bandwidth share that delays stripe 1 by ~5us) nor lands
            # as one full-bandwidth burst during peak prefix compute (which
            # trips the P0 power downclock for the rest of the run -
            # measured 238us vs 203).
            Wpt = const.tile([P, KC, C], F16, name="Wpt")
            Wp = [Wpt[:, kc, :] for kc in range(KC)]
            bp_bcast = const.tile([P, C], F32)

            def emit_late_loads(g):
                if g != 0:
                    return
                HW = SW // 2
                for gg in range(NS):
                    for kc in range(KC):
                        for hh in range(2):
                            sl = slice(gg * SW + hh * HW, gg * SW + (hh + 1) * HW)
                            nc.gpsimd.dma_start(F1t[:, kc, sl], dF1v[:, kc, sl])
                nc.gpsimd.dma_start(ones_col, d_ones_col[:])
                nc.gpsimd.dma_start(bp_row, dBp[:])
                nc.gpsimd.dma_start(Wpt, chunked_dram(dWp[:, :], C))
                nc.gpsimd.partition_broadcast(bp_bcast, bp_row)

            # ---- prefix: kvT projections + kvn transposes (dense PE) ----
            with tc.tile_pool(name="pfps", bufs=8, space="PSUM") as pfps:
                for g in range(NS):
                    # kvT stripe g for all 4 output chunks
                    for co in range(KC):
                        pj = pfps.tile([P, SW], F32, tag="pj", bufs=4)
                        for kc in range(KC):
                            nc.tensor.matmul(
                                pj,
                                W[kc][:, co * P:(co + 1) * P],
                                FT2[kc][:, g * SW:(g + 1) * SW],
                                start=(kc == 0),
                                stop=(kc == KC - 1),
                            )
                        # evac on ACT (idle in prefix): kvT = pj + bq
                        nc.scalar.activation(
                            kvT[co][:, g * SW:(g + 1) * SW],
                            pj,
                            IDENT,
                            bias=bq_col[:, co:co + 1],
                        )
                        if co == 0:
                            emit_late_loads(g)
                    # kvn for this stripe's 4 m-blocks
                    for mb in range(4 * g, 4 * g + 4):
                        pjt = pfps.tile([P, C], F16, tag="pjt", bufs=2)
                        for hh in range(H):
                            nc.tensor.transpose(
                                pjt[:, hh * P:(hh + 1) * P],
                                kvT[hh][:, mb * P:(mb + 1) * P],
                                ident16,
                            )
                        if PV_FP8:
                            with nc.allow_low_precision(
                                reason="fp8 kv for DoubleRow pv; metric "
                                "1.3e-2 vs 2e-2 gate (measured end-to-end)"
                            ):
                                nc.vector.tensor_copy(
                                    kvn2[mb // 2][:, mb % 2, :], pjt
                                )
                        else:
                            nc.vector.tensor_copy(kvn[mb], pjt)

            # ---- attention + weaved qT projections + weaved out-proj ----
            with (
                tc.tile_pool(name="xtp", bufs=1) as xtp,
                tc.tile_pool(name="et", bufs=2) as epool,
                tc.tile_pool(name="es", bufs=2) as espool,
                tc.tile_pool(name="scps", bufs=2, space="PSUM") as scps,
                tc.tile_pool(name="pvps", bufs=2, space="PSUM") as pvps,
                tc.tile_pool(name="auxps", bufs=2, space="PSUM") as auxps,
                tc.tile_pool(name="sm", bufs=2) as sm,
                tc.tile_pool(name="osb", bufs=4) as osb,
            ):
                xT = [xtp.tile([P, N], F16, name=f"xT{i}") for i in range(KC)]

                def emit_qT_proj(co, g):
                    pj = auxps.tile([P, SW], F32, tag="aux")
                    for kc in range(KC):
                        nc.tensor.matmul(
                            pj,
                            W[kc][:, co * P:(co + 1) * P],
                            FT1[kc][:, g * SW:(g + 1) * SW],
                            start=(kc == 0),
                            stop=(kc == KC - 1),
                        )
                    nc.vector.tensor_scalar_add(
                        qT[co][:, g * SW:(g + 1) * SW],
                        pj,
                        bq_col[:, co:co + 1],
                    )

                def emit_ph4_start(nb, nchunks=KC, pool=None):
                    if pool is None:
                        pr = auxps.tile([P, C], F32, tag="aux", name="pr")
                    else:
                        # tail only: borrow a freed sc-ring slot (2 banks,
                        # use one slab) so the aux ring stays available
                        pr = pool.tile([P, 2, SW], F32, tag="sc", name="prt")[
                            :, 0, :
                        ]
                    for kc in range(nchunks):
                        nc.tensor.matmul(
                            pr,
                            xT[kc][:, nb * P:(nb + 1) * P],
                            Wp[kc],
                            start=(kc == 0),
                            stop=(kc == KC - 1),
                        )
                    return pr

                def emit_ph4_finish(nb, pr, kc0=KC):
                    for kc in range(kc0, KC):
                        nc.tensor.matmul(
                            pr,
                            xT[kc][:, nb * P:(nb + 1) * P],
                            Wp[kc],
                            start=False,
                            stop=(kc == KC - 1),
                        )
                    ot = osb.tile([P, C], F32, tag="ot")
                    nc.vector.tensor_add(ot, pr, bp_bcast)
                    nc.sync.dma_start(dOut[nb * P:(nb + 1) * P, :], ot)

                def emit_ph4_nb(nb):
                    emit_ph4_finish(nb, emit_ph4_start(nb))

                combos = [(s, h) for s in range(NS) for h in range(H)]
                # qT-proj weave (j1 slot): combo k emits combo k+1's qT
                qt_sched = [None] * 16
                for k in range(15):
                    qt_sched[k] = combos[k + 1]
                # out-proj weave: stripe s's 4 blocks at combos 4(s+1)+1
                # (j4+j6) and 4(s+1)+2 (j4+j6); stripe 3 in the tail
                op_sched = [[] for _ in range(16)]
                for s in range(NS - 1):
                    for i, nb in enumerate(range(4 * s, 4 * s + 4)):
                        op_sched[4 * (s + 1) + 1 + i // 2].append(nb)

                # deferred normalize chain state from the previous combo
                pending = {}

                def chain_reduce(pp, tail=False):
                    if DN_MODE == "gpsimd":
                        nc.gpsimd.partition_all_reduce(
                            pp["dnb"], pp["esE"], channels=P,
                            reduce_op=bass.bass_isa.ReduceOp.add,
                        )
                        return
                    # in the tail the aux ring is full of out-proj partials;
                    # the freed pv ring hosts the last chain's psum row
                    if tail:
                        ct = pvps.tile([P, SW], F32, tag="pv", name="ct")
                    else:
                        ct = auxps.tile([P, C], F32, tag="aux", name="ct")
                    pp["chain"] = ct
                    nc.tensor.matmul(
                        ct[0:1, 0:SW], ones_col, pp["esE"],
                        start=True, stop=True,
                    )

                def chain_recip(pp):
                    if DN_MODE == "gpsimd":
                        nc.vector.reciprocal_approx_fast(pp["recip"], pp["dnb"])
                        return
                    nc.vector.reciprocal_approx_fast(
                        pp["rrow"], pp["chain"][0:1, 0:SW]
                    )

                def chain_bcast(pp):
                    if DN_MODE == "gpsimd":
                        return
                    # small gpsimd op (~0.7us): 16x cheaper than v3's
                    # partition_all_reduce of the full [P,SW] tile
                    nc.gpsimd.partition_broadcast(pp["bcast"], pp["rrow"])

                def chain_mul(pp):
                    s, h = pp["sh"]
                    mulin = pp["recip"] if DN_MODE == "gpsimd" else pp["bcast"]
                    with nc.allow_low_precision(
                        reason="x values O(0.1); fp16 keeps 5e-4 rel"
                    ):
                        nc.vector.tensor_mul(
                            xT[h][:, s * SW:(s + 1) * SW],
                            pp["pv"], mulin,
                        )

                pvq = deque()
                emit_qT_proj(0, 0)  # combo 0's qT, ahead of the loop

                HP = P // 2  # 64: DoubleRow output-partition limit
                HS = SW // 2  # 256: DoubleRow moving-free limit

                for k, (s, h) in enumerate(combos):
                    E = epool.tile([P, MB, SW], F8 if PV_FP8 else F16, tag="E")
                    pv = pvps.tile([P, SW], F32, tag="pv")

                    if PV_FP8:
                        def pv_pair(jj, E=E, pv=pv, h=h):
                            # 4 DoubleRow matmuls: out quadrant [64, 256],
                            # contraction 256 (m-blocks 2jj, 2jj+1)
                            for dh in range(2):
                                for nh in range(2):
                                    nc.tensor.matmul(
                                        pv[dh * HP:(dh + 1) * HP,
                                           nh * HS:(nh + 1) * HS],
                                        kvn2[jj][:, :,
                                                 h * P + dh * HP:
                                                 h * P + (dh + 1) * HP],
                                        E[:, 2 * jj:2 * jj + 2,
                                          nh * HS:(nh + 1) * HS],
                                        start=(jj == 0),
                                        stop=(jj == MB // 2 - 1),
                                        perf_mode=DR,
                                        tile_position=(0, dh * HP),
                                    )
                    else:
                        def pv_pair(jj, E=E, pv=pv, h=h):
                            for mb in (2 * jj, 2 * jj + 1):
                                nc.tensor.matmul(
                                    pv,
                                    kvn[mb][:, h * P:(h + 1) * P],
                                    E[:, mb, :],
                                    start=(mb == 0),
                                    stop=(mb == MB - 1),
                                )

                    esA = espool.tile([P, 4, SW], F16, tag="esA")
                    esB = espool.tile([P, 4, SW], F16, tag="esB")
                    esC = espool.tile([P, 4, SW], F16, tag="esC")
                    esD = espool.tile([P, 2, SW], F16, tag="esD")
                    esE = espool.tile([P, SW], F16, tag="esE")
                    for j in range(MB // 2):
                        sc = scps.tile([P, 2, SW], F32, tag="sc")
                        for i in range(2):
                            mb = 2 * j + i
                            nc.tensor.matmul(
                                sc[:, i, :],
                                kvT[h][:, mb * P:(mb + 1) * P],
                                qT[h][:, s * SW:(s + 1) * SW],
                                start=True,
                                stop=True,
                            )
                        with nc.allow_low_precision(
                            reason="fp8 E for DoubleRow pv; metric 1.3e-2 "
                            "vs 2e-2 gate (measured end-to-end)"
                        ):
                            nc.scalar.activation(
                                E[:, 2 * j:2 * j + 2, :].rearrange(
                                    "p a b -> p (a b)"
                                ),
                                sc.rearrange("p a b -> p (a b)"),
                                EXP,
                                scale=float(SCALE),
                            )
                        pvq.append(lambda jj=j, pf=pv_pair: pf(jj))
                        if len(pvq) > 2:
                            pvq.popleft()()
                        # weave + deferred chain of the previous combo.
                        # progressive denominator tree: esA@j4, +E[8:12]@j6,
                        # +E[12:16] after the loop, so esE trails the last
                        # exp by only ~2 DVE ops (the v4 bulk tree made the
                        # next combo's chain_reduce stall the PE queue head).
                        if j == 1 and qt_sched[k] is not None:
                            g2, h2 = qt_sched[k][0], qt_sched[k][1]
                            emit_qT_proj(h2, g2)
                        elif j == 3 and pending:
                            chain_reduce(pending)
                        elif j == 4:
                            if pending:
                                chain_recip(pending)
                                chain_bcast(pending)
                            if op_sched[k]:
                                emit_ph4_nb(op_sched[k][0])
                            with nc.allow_low_precision(
                                reason="fp16 partial sums of E for softmax "
                                "denominator; ~1e-3 rel"
                            ):
                                # fp8 inputs lose the DVE 2-byte perf mode;
                                # the first (widest) level goes to gpsimd,
                                # which is otherwise idle in the main loop
                                eng = nc.gpsimd if PV_FP8 else nc.vector
                                eng.tensor_add(
                                    esA, E[:, 0:4, :], E[:, 4:8, :]
                                )
                        elif j == 5 and pending:
                            chain_mul(pending)
                        elif j == 6:
                            with nc.allow_low_precision(
                                reason="fp16 partial sums of E for softmax "
                                "denominator; ~1e-3 rel"
                            ):
                                nc.vector.tensor_add(esB, esA, E[:, 8:12, :])
                            if len(op_sched[k]) > 1:
                                emit_ph4_nb(op_sched[k][1])
                    with nc.allow_low_precision(
                        reason="fp16 partial sums of E for softmax "
                        "denominator; ~1e-3 rel"
                    ):
                        nc.vector.tensor_add(esC, esB, E[:, 12:16, :])
                        nc.vector.tensor_add(esD, esC[:, 0:2, :], esC[:, 2:4, :])
                        nc.vector.tensor_add(esE, esD[:, 0, :], esD[:, 1, :])
                    pending = {"sh": (s, h), "pv": pv, "esE": esE}
                    if DN_MODE == "gpsimd":
                        pending["dnb"] = sm.tile(
                            [P, SW], F32, tag="dnb", name="dnb"
                        )
                        pending["recip"] = sm.tile(
                            [P, SW], F32, tag="recip", name="recip"
                        )
                    else:
                        pending["rrow"] = sm.tile(
                            [1, SW], F32, tag="rrow", name="rrow"
                        )
                        pending["bcast"] = sm.tile(
                            [P, SW], F32, tag="bcast", name="bcast"
                        )

                # ---- tail: drain pv, last chain, last stripe's out-proj.
                # nb12/nb13's first 3 chunks (heads whose chains are already
                # normalized) fill the PE while the chain-15 latency runs;
                # their h=3 chunks wait on chain_mul.  The chain psum lives
                # in the freed pv ring so both aux banks hold partials. ----
                while pvq:
                    pvq.popleft()()
                pr12 = emit_ph4_start(12, nchunks=KC - 1)
                pr13 = emit_ph4_start(13, nchunks=KC - 1)
                chain_reduce(pending, tail=True)
                # nb14/nb15 partials borrow the freed sc ring
                pr14 = emit_ph4_start(14, nchunks=KC - 1, pool=scps)
                pr15 = emit_ph4_start(15, nchunks=KC - 1, pool=scps)
                chain_recip(pending)
                chain_bcast(pending)
                chain_mul(pending)
                emit_ph4_finish(12, pr12, kc0=KC - 1)
                emit_ph4_finish(13, pr13, kc0=KC - 1)
                emit_ph4_finish(14, pr14, kc0=KC - 1)
                emit_ph4_finish(15, pr15, kc0=KC - 1)

    nc.compile()
    return nc


_NC = None


def _get_nc():
    global _NC
    if _NC is None:
        _NC = build_nc()
    return _NC


def kernel(F1, F2, W_qkv, b_qkv, W_proj, b_proj, _trace=False):
    F1 = np.asarray(F1)
    F2 = np.asarray(F2)
    F1T = np.ascontiguousarray(
        F1.astype(np.float16).transpose(0, 2, 1)
    )  # [B, C, N]
    F2T = np.ascontiguousarray(F2.astype(np.float16).transpose(0, 2, 1))
    Wh = np.ascontiguousarray(np.asarray(W_qkv).astype(np.float16))
    Wph = np.ascontiguousarray(np.asarray(W_proj).astype(np.float16))
    bqc = np.ascontiguousarray(
        np.asarray(b_qkv, dtype=np.float32).reshape(KC, P).T
    )
    bph = np.ascontiguousarray(
        np.asarray(b_proj, dtype=np.float32).reshape(1, C)
    )

    nc = _get_nc()
    in_maps = [
        {"F1T": F1T[b], "F2T": F2T[b], "Wqkv": Wh, "bqc": bqc,
         "Wproj": Wph, "bproj": bph}
        for b in range(B)
    ]
    res = run_bass_kernel_spmd(
        nc, in_maps, core_ids=list(range(B)), trace=_trace
    )
    out = np.stack([res.results[b]["OUT"] for b in range(B)], axis=0)
    if _trace:
        return out, res
    return out
